# revision 20
# baseline (speedup 1.0000x reference)
"""Cross-attention kernel for Trainium2, sharded over 8 NeuronCores.

Problem (hardcoded shapes): B=2, N=4096, M=1024, DIM=1024, H=16, D=64.
  q = rms_norm(x @ Wq.T + bq)        per-head, gamma gq, eps 1e-6
  k = rms_norm(ctx @ Wk.T + bk)      (Wk = first half of Wkv)
  v = ctx @ Wv.T + bv                (Wv = second half of Wkv)
  out = softmax(q k^T / sqrt(D) + mask_bias) @ v
  y = out @ Wo.T + bo

Sharding: tensor-parallel over the 16 heads -> 2 heads per core.
Each core computes q/k/v projections for its 2 heads (column-sharded
Wq/Wkv), full attention for those heads, and a partial output
projection (row-sharded Wo).  The host sums the 8 partial outputs.

v3 design notes (vs the 609us baseline):
 - All matmul operands are bf16 (FWL halves LDWEIGHTS, DMA halves).
   PSUM stays f32; normalization math in f32.
 - Biases are seeded into PSUM with a [1,128] bias-row matmul against
   a ones vector, so no DVE pass is needed to add them.
 - rms_norm: per-head sum-of-squares via indicator matmul, sqrt on
   ACT, reciprocal on DVE, per-head broadcast on GPSIMD, and one
   scalar_tensor_tensor per head applies gamma*rstd straight out of
   PSUM.
 - Attention epilogue reciprocal reads PSUM directly; the out-proj
   matmuls of chunk nt are emitted after chunk nt+1's last PV matmul
   so the in-order PE queue always has ready work while the epilogue
   chain (DVE recip -> GPSIMD bcast -> DVE muls) completes.
 - Softmax denominator rides as a 65th column of V (ones, masked).
 - y is written as bf16 partials; the host sums the 8 cores.
"""

import numpy as np

P = 128
B = 2
N = 4096
M = 1024
C = 1024  # DIM == COND_DIM
H = 16
D = 64
HC = 2  # heads per core
CC = C // P  # contraction chunks
NT = N // 1024  # query chunks of 1024 (projection granularity)
NA = N // 512  # query chunks of 512 (attention granularity)
MC = M // P  # kv chunks of 128
VA = D + 1  # V columns + denominator column
EPS = 1e-6

_CACHE = {}


def _build():
    if "nc" in _CACHE:
        return _CACHE["nc"]

    import concourse.bass as bass  # noqa: F401
    import concourse.tile as tile
    from concourse import bacc, mybir

    f32 = mybir.dt.float32
    bf16 = mybir.dt.bfloat16
    AF = mybir.ActivationFunctionType
    MUL = mybir.AluOpType.mult

    nc = bacc.Bacc("TRN2", target_bir_lowering=False, debug=False, num_devices=8)

    xt_d = nc.dram_tensor("xt", [B, C, N], bf16, kind="ExternalInput").ap()
    ctxt_d = nc.dram_tensor("ctxt", [B, C, M], bf16, kind="ExternalInput").ap()
    wqt_d = nc.dram_tensor("wqt", [C, P], bf16, kind="ExternalInput").ap()
    wkt_d = nc.dram_tensor("wkt", [C, P], bf16, kind="ExternalInput").ap()
    wvt_d = nc.dram_tensor("wvt", [C, P], bf16, kind="ExternalInput").ap()
    wot_d = nc.dram_tensor("wot", [P, C], bf16, kind="ExternalInput").ap()
    brow_d = nc.dram_tensor("brow", [1, 3 * P], bf16, kind="ExternalInput").ap()
    gq_d = nc.dram_tensor("gqcol", [D, HC], f32, kind="ExternalInput").ap()
    gk_d = nc.dram_tensor("gkcol", [D, HC], f32, kind="ExternalInput").ap()
    ind2_d = nc.dram_tensor("ind2", [P, VA], bf16, kind="ExternalInput").ap()
    ident_d = nc.dram_tensor("ident", [P, P], f32, kind="ExternalInput").ap()
    mask_d = nc.dram_tensor("maskf", [P, B * MC], f32, kind="ExternalInput").ap()
    y_d = nc.dram_tensor("y", [B, N, C], bf16, kind="ExternalOutput").ap()

    with tile.TileContext(nc) as tc:
        with (
            tc.tile_pool(name="consts", bufs=1) as consts,
            tc.tile_pool(name="ctxp", bufs=2) as ctxp,
            tc.tile_pool(name="xp", bufs=10) as xp,
            tc.tile_pool(name="qk", bufs=1) as qk,
            tc.tile_pool(name="work", bufs=2) as work,
            tc.tile_pool(name="ptp", bufs=3) as ptp,
            tc.tile_pool(name="ps", bufs=1, space="PSUM") as ps,
        ):
            wq_sb = consts.tile([P, CC, P], bf16)
            nc.sync.dma_start(wq_sb[:], wqt_d.rearrange("(o p) m -> p o m", p=P))
            brow_sb = consts.tile([1, 3 * P], bf16)
            nc.sync.dma_start(brow_sb[:], brow_d[:])
            gq_sb = consts.tile([D, HC], f32)
            nc.sync.dma_start(gq_sb[:], gq_d[:])
            ind2_sb = consts.tile([P, VA], bf16)
            nc.sync.dma_start(ind2_sb[:], ind2_d[:])
            wk_sb = consts.tile([P, CC, P], bf16)
            nc.sync.dma_start(wk_sb[:], wkt_d.rearrange("(o p) m -> p o m", p=P))
            wv_sb = consts.tile([P, CC, P], bf16)
            nc.sync.dma_start(wv_sb[:], wvt_d.rearrange("(o p) m -> p o m", p=P))
            wo_sb = consts.tile([P, C], bf16)
            nc.sync.dma_start(wo_sb[:], wot_d[:])
            gk_sb = consts.tile([D, HC], f32)
            nc.sync.dma_start(gk_sb[:], gk_d[:])
            ident_sb = consts.tile([P, P], f32)
            nc.sync.dma_start(ident_sb[:], ident_d[:])
            mask_sb = consts.tile([P, B * MC], f32)
            nc.sync.dma_start(mask_sb[:], mask_d[:])
            eps_sb = consts.tile([VA, 1], f32)
            nc.vector.memset(eps_sb[:], EPS)
            ones_sb = consts.tile([1, 512], bf16)
            nc.vector.memset(ones_sb[:], 1.0)

            def proj(ps_dst, w_sb, src, brow):
                """ps_dst[128,1024] (psum) = w^T src + bias (seeded)."""
                for half in range(2):
                    hs = slice(half * 512, (half + 1) * 512)
                    nc.tensor.matmul(
                        ps_dst[:, hs], brow, ones_sb[:], start=True, stop=False
                    )
                    for cc in range(CC):
                        mv = src[cc][:, hs] if isinstance(src, list) else src[:, cc, hs]
                        nc.tensor.matmul(
                            ps_dst[:, hs],
                            w_sb[:, cc],
                            mv,
                            start=False,
                            stop=(cc == CC - 1),
                        )

            def rms_norm_chunk(psrc, gcol, dst):
                """psrc: [128, 1024] psum of biased raw projections for 2
                heads stacked [64|64]; writes normalized bf16 to dst.
                The two raw copies free the PSUM tile right away; the rest
                of the chain runs out of SBUF."""
                raw_a = work.tile([D, 1024], bf16, tag="raw_a", name="raw_a")
                nc.vector.tensor_copy(raw_a[:], psrc[0:D, :])
                raw_b = work.tile([D, 1024], bf16, tag="raw_b", name="raw_b")
                nc.vector.tensor_copy(raw_b[:], psrc[D:P, :])
                sq = work.tile([P, 1024], bf16, tag="sq", name="sq")
                nc.vector.tensor_mul(sq[0:D, :], raw_a[:], raw_a[:])
                nc.vector.tensor_mul(sq[D:P, :], raw_b[:], raw_b[:])
                ss = ps.tile([VA, 1024], f32, tag="O", bufs=2, name="ss")
                for half in range(2):
                    hs = slice(half * 512, (half + 1) * 512)
                    nc.tensor.matmul(
                        ss[:, hs], ind2_sb[:], sq[:, hs], start=True, stop=True
                    )
                srt = work.tile([VA, 1024], f32, tag="srt", name="srt", bufs=1)
                nc.scalar.activation(
                    srt[:], ss[:], AF.Sqrt, scale=1.0 / D, bias=eps_sb[:]
                )
                rstd = work.tile([VA, 1024], f32, tag="rstd", name="rstd", bufs=1)
                nc.vector.reciprocal_approx_fast(out=rstd[:], in_=srt[:])
                r1 = work.tile([1, 1024], f32, tag="r1", name="r1", bufs=1)
                nc.vector.tensor_copy(r1[:], rstd[D : D + 1, :])
                rbc0 = work.tile([D, 1024], f32, tag="rbc0", name="rbc0", bufs=1)
                nc.gpsimd.partition_broadcast(rbc0[:], rstd[0:1, :])
                rbc1 = work.tile([D, 1024], f32, tag="rbc1", name="rbc1", bufs=1)
                nc.gpsimd.partition_broadcast(rbc1[:], r1[:])
                nc.vector.scalar_tensor_tensor(
                    out=dst[0:D, :], in0=raw_a[:], scalar=gcol[:, 0:1],
                    in1=rbc0[:], op0=MUL, op1=MUL,
                )
                nc.vector.scalar_tensor_tensor(
                    out=dst[D:P, :], in0=raw_b[:], scalar=gcol[:, 1:2],
                    in1=rbc1[:], op0=MUL, op1=MUL,
                )

            qtn = [
                qk.tile([P, 1024], bf16, tag=f"qtn{i}", name=f"qtn{i}")
                for i in range(B * NT)
            ]
            outp = qk.tile([P, N], bf16, tag="outtn")  # per-batch, reused

            def outproj_chunk(b, tc_):
                y_sb = work.tile([P, C], bf16, tag="ysb", bufs=3)
                ps_y = ps.tile([P, 1024], f32, tag="A", bufs=2)
                for ec in range(2):
                    nc.tensor.matmul(
                        ps_y[:, ec * 512 : (ec + 1) * 512],
                        outp[:, tc_ * P : (tc_ + 1) * P],
                        wo_sb[:, ec * 512 : (ec + 1) * 512],
                        start=True,
                        stop=True,
                    )
                if tc_ % 2 == 0:
                    nc.vector.tensor_copy(y_sb[:], ps_y[:])
                else:
                    nc.scalar.activation(y_sb[:], ps_y[:], AF.Copy)
                nc.sync.dma_start(y_d[b, tc_ * P : (tc_ + 1) * P, :], y_sb[:])

            for b in range(B):
                # ---- Q projection; ctx prefetch after the first chunk's
                # xt DMAs so the PE starts as early as possible ----
                ctx_sb = None
                xt_r = xt_d[b].rearrange("(o p) n -> p o n", p=P)
                for nt in range(NT):
                    xt_cc = []
                    for cc in range(CC):
                        t = xp.tile([P, 1024], bf16, tag="xt")
                        nc.sync.dma_start(
                            t[:], xt_r[:, cc, nt * 1024 : (nt + 1) * 1024]
                        )
                        xt_cc.append(t)
                    if ctx_sb is None:
                        ctx_sb = ctxp.tile([P, CC, M], bf16, tag="ctx")
                        nc.sync.dma_start(
                            ctx_sb[:], ctxt_d[b].rearrange("(o p) m -> p o m", p=P)
                        )
                    ps_q = ps.tile([P, 1024], f32, tag="A", bufs=2)
                    proj(ps_q, wq_sb, xt_cc, brow_sb[:, 0:P])
                    rms_norm_chunk(ps_q[:], gq_sb, qtn[b * NT + nt][:])

                # ---- KV phase ----
                ktn = qk.tile([P, M], bf16, tag="ktn", bufs=2)
                ps_k = ps.tile([P, 1024], f32, tag="A", bufs=2)
                proj(ps_k, wk_sb, ctx_sb, brow_sb[:, P : 2 * P])
                rms_norm_chunk(ps_k[:], gk_sb, ktn[:])

                ps_v = ps.tile([P, 1024], f32, tag="A", bufs=2)
                proj(ps_v, wv_sb, ctx_sb, brow_sb[:, 2 * P : 3 * P])
                vt_sb = work.tile([P, M], f32, tag="vt", bufs=2)
                nc.vector.tensor_copy(vt_sb[:], ps_v[:])
                vaug = qk.tile([P, MC, 2 * VA], bf16, tag="vaug", bufs=2)
                for mc in range(MC):
                    ps_t = ps.tile([P, P], f32, tag="O", bufs=2)
                    nc.tensor.transpose(
                        ps_t[:], vt_sb[:, mc * P : (mc + 1) * P], ident_sb[:]
                    )
                    mcol = mask_sb[:, b * MC + mc : b * MC + mc + 1]
                    nc.vector.tensor_mul(
                        vaug[:, mc, 0:D], ps_t[:, 0:D], mcol.to_broadcast((P, D))
                    )
                    nc.vector.tensor_copy(vaug[:, mc, D : D + 1], mcol)
                    nc.vector.tensor_mul(
                        vaug[:, mc, VA : VA + D],
                        ps_t[:, D : 2 * D],
                        mcol.to_broadcast((P, D)),
                    )
                    nc.vector.tensor_copy(vaug[:, mc, VA + D : VA + D + 1], mcol)

                # ---- attention; out-proj of chunk nt-1 rides behind ----
                for na in range(NA):
                    nsl = slice(na * 512, (na + 1) * 512)
                    qt = qtn[b * NT + na // 2]
                    qsl = slice((na % 2) * 512, (na % 2) * 512 + 512)
                    ps_o = ps.tile([VA, 1024], f32, tag="O", bufs=2, name="ps_o")
                    for mc in range(MC):
                        msl = slice(mc * P, (mc + 1) * P)
                        ps_s = ps.tile([P, 1024], f32, tag="A", bufs=2)
                        for h in range(2):
                            hsl = slice(D * h, D * (h + 1))
                            nc.tensor.matmul(
                                ps_s[:, h * 512 : (h + 1) * 512],
                                ktn[hsl, msl],
                                qt[hsl, qsl],
                                start=True,
                                stop=True,
                            )
                        pt = ptp.tile([P, 1024], bf16, tag="pt")
                        nc.scalar.activation(pt[:], ps_s[:], AF.Exp)
                        for h in range(2):
                            nc.tensor.matmul(
                                ps_o[:, h * 512 : (h + 1) * 512],
                                vaug[:, mc, h * VA : (h + 1) * VA],
                                pt[:, h * 512 : (h + 1) * 512],
                                start=(mc == 0),
                                stop=(mc == MC - 1),
                            )
                    # epilogue: divide by the denominator row (64) per head
                    den = work.tile([1, 1024], f32, tag="den", name="den", bufs=1)
                    nc.vector.tensor_copy(den[:], ps_o[D : D + 1, :])
                    rec = work.tile([1, 1024], f32, tag="rec", name="rec", bufs=1)
                    nc.vector.reciprocal_approx_fast(out=rec[:], in_=den[:])
                    rbc2 = work.tile([D, 1024], f32, tag="rbc2", name="rbc2", bufs=1)
                    nc.gpsimd.partition_broadcast(rbc2[:], rec[:])
                    nc.vector.scalar_tensor_tensor(
                        out=outp[0:D, nsl], in0=ps_o[0:D, 0:512], scalar=1.0,
                        in1=rbc2[:, 0:512], op0=MUL, op1=MUL,
                    )
                    nc.vector.scalar_tensor_tensor(
                        out=outp[D:P, nsl], in0=ps_o[0:D, 512:1024], scalar=1.0,
                        in1=rbc2[:, 512:1024], op0=MUL, op1=MUL,
                    )

                # ---- output projection phase (PE-dense) ----
                for tc_ in range(N // P):
                    outproj_chunk(b, tc_)


    nc.compile()
    _CACHE["nc"] = nc
    return nc


def _make_in_maps(x, context, context_mask, Wq, bq, Wkv, bkv, gq, gk, Wo, bo):
    import ml_dtypes

    f32 = np.float32
    bf16 = ml_dtypes.bfloat16
    xt = np.ascontiguousarray(np.transpose(x, (0, 2, 1))).astype(bf16)
    ctxt = np.ascontiguousarray(np.transpose(context, (0, 2, 1))).astype(bf16)
    # maskf[p, b*MC + mc] = mask[b, mc*128 + p]
    maskf = np.ascontiguousarray(
        np.transpose(
            np.asarray(context_mask, dtype=f32).reshape(B, MC, P), (2, 0, 1)
        ).reshape(P, B * MC)
    )
    ident = np.eye(P, dtype=f32)
    ind2 = np.zeros((P, VA), dtype=f32)
    ind2[0:D, 0] = 1.0
    ind2[D : 2 * D, D] = 1.0
    ind2 = ind2.astype(bf16)

    in_maps = []
    for c in range(8):
        hs = slice(P * c, P * (c + 1))
        gqcol = np.zeros((D, HC), dtype=f32)
        gkcol = np.zeros((D, HC), dtype=f32)
        for h in range(HC):
            gqcol[:, h] = gq[HC * c + h] * (1.0 / np.sqrt(D))
            gkcol[:, h] = gk[HC * c + h]
        brow = np.zeros((1, 3 * P), dtype=f32)
        brow[0, 0:P] = np.asarray(bq[hs], dtype=f32)
        brow[0, P : 2 * P] = np.asarray(bkv[hs], dtype=f32)
        brow[0, 2 * P : 3 * P] = np.asarray(
            bkv[C + P * c : C + P * (c + 1)], dtype=f32
        )
        in_maps.append(
            {
                "xt": xt,
                "ctxt": ctxt,
                "wqt": np.ascontiguousarray(Wq[hs].T).astype(bf16),
                "wkt": np.ascontiguousarray(Wkv[hs].T).astype(bf16),
                "wvt": np.ascontiguousarray(
                    Wkv[C + P * c : C + P * (c + 1)].T
                ).astype(bf16),
                "wot": np.ascontiguousarray(Wo[:, hs].T).astype(bf16),
                "brow": brow.astype(bf16),
                "gqcol": gqcol,
                "gkcol": gkcol,
                "ind2": ind2,
                "ident": ident,
                "maskf": maskf,
            }
        )
    return in_maps


def _run(in_maps, **spmd_kwargs):
    from concourse import bass_utils

    nc = _build()
    return bass_utils.run_bass_kernel_spmd(
        nc, in_maps, core_ids=list(range(8)), **spmd_kwargs
    )


def kernel(x, context, context_mask, Wq, bq, Wkv, bkv, gq, gk, Wo, bo):
    in_maps = _make_in_maps(
        x, context, context_mask, Wq, bq, Wkv, bkv, gq, gk, Wo, bo
    )
    res = _run(in_maps)
    y = np.zeros((B, N, C), dtype=np.float64)
    for c in range(8):
        y += np.asarray(res.results[c]["y"], dtype=np.float64)
    y += np.asarray(bo, dtype=np.float64)
    return y.astype(np.float32)


# revision 22
# speedup vs baseline: 1.0083x; 1.0083x over previous
"""Cross-attention kernel for Trainium2, sharded over 8 NeuronCores.

Problem (hardcoded shapes): B=2, N=4096, M=1024, DIM=1024, H=16, D=64.
  q = rms_norm(x @ Wq.T + bq)        per-head, gamma gq, eps 1e-6
  k = rms_norm(ctx @ Wk.T + bk)      (Wk = first half of Wkv)
  v = ctx @ Wv.T + bv                (Wv = second half of Wkv)
  out = softmax(q k^T / sqrt(D) + mask_bias) @ v
  y = out @ Wo.T + bo

Sharding: tensor-parallel over the 16 heads -> 2 heads per core.
Each core computes q/k/v projections for its 2 heads (column-sharded
Wq/Wkv), full attention for those heads, and a partial output
projection (row-sharded Wo).  The host sums the 8 partial outputs.

v3 design notes (vs the 609us baseline):
 - All matmul operands are bf16 (FWL halves LDWEIGHTS, DMA halves).
   PSUM stays f32; normalization math in f32.
 - Biases are seeded into PSUM with a [1,128] bias-row matmul against
   a ones vector, so no DVE pass is needed to add them.
 - rms_norm: per-head sum-of-squares via indicator matmul, sqrt on
   ACT, reciprocal on DVE, per-head broadcast on GPSIMD, and one
   scalar_tensor_tensor per head applies gamma*rstd straight out of
   PSUM.
 - Attention epilogue reciprocal reads PSUM directly; the out-proj
   matmuls of chunk nt are emitted after chunk nt+1's last PV matmul
   so the in-order PE queue always has ready work while the epilogue
   chain (DVE recip -> GPSIMD bcast -> DVE muls) completes.
 - Softmax denominator rides as a 65th column of V (ones, masked).
 - y is written as bf16 partials; the host sums the 8 cores.
"""

import numpy as np

P = 128
B = 2
N = 4096
M = 1024
C = 1024  # DIM == COND_DIM
H = 16
D = 64
HC = 2  # heads per core
CC = C // P  # contraction chunks
NT = N // 1024  # query chunks of 1024 (projection granularity)
NA = N // 512  # query chunks of 512 (attention granularity)
MC = M // P  # kv chunks of 128
VA = D + 1  # V columns + denominator column
EPS = 1e-6

_CACHE = {}


def _build():
    if "nc" in _CACHE:
        return _CACHE["nc"]

    import concourse.bass as bass  # noqa: F401
    import concourse.tile as tile
    from concourse import bacc, mybir

    f32 = mybir.dt.float32
    bf16 = mybir.dt.bfloat16
    AF = mybir.ActivationFunctionType
    MUL = mybir.AluOpType.mult

    nc = bacc.Bacc("TRN2", target_bir_lowering=False, debug=False, num_devices=8)

    xt_d = nc.dram_tensor("xt", [B, C, N], bf16, kind="ExternalInput").ap()
    ctxt_d = nc.dram_tensor("ctxt", [B, C, M], bf16, kind="ExternalInput").ap()
    wqt_d = nc.dram_tensor("wqt", [C, P], bf16, kind="ExternalInput").ap()
    wkt_d = nc.dram_tensor("wkt", [C, P], bf16, kind="ExternalInput").ap()
    wvt_d = nc.dram_tensor("wvt", [C, P], bf16, kind="ExternalInput").ap()
    wot_d = nc.dram_tensor("wot", [P, C], bf16, kind="ExternalInput").ap()
    brow_d = nc.dram_tensor("brow", [1, 3 * P], bf16, kind="ExternalInput").ap()
    gq_d = nc.dram_tensor("gqcol", [D, HC], f32, kind="ExternalInput").ap()
    gk_d = nc.dram_tensor("gkcol", [D, HC], f32, kind="ExternalInput").ap()
    ind2_d = nc.dram_tensor("ind2", [P, VA], bf16, kind="ExternalInput").ap()
    ident_d = nc.dram_tensor("ident", [P, P], f32, kind="ExternalInput").ap()
    mask_d = nc.dram_tensor("maskf", [P, B * MC], f32, kind="ExternalInput").ap()
    y_d = nc.dram_tensor("y", [B, N, C], bf16, kind="ExternalOutput").ap()

    with tile.TileContext(nc) as tc:
        with (
            tc.tile_pool(name="consts", bufs=1) as consts,
            tc.tile_pool(name="ctxp", bufs=2) as ctxp,
            tc.tile_pool(name="xp", bufs=10) as xp,
            tc.tile_pool(name="qk", bufs=1) as qk,
            tc.tile_pool(name="work", bufs=2) as work,
            tc.tile_pool(name="ptp", bufs=3) as ptp,
            tc.tile_pool(name="ps", bufs=1, space="PSUM") as ps,
        ):
            wq_sb = consts.tile([P, CC, P], bf16)
            nc.sync.dma_start(wq_sb[:], wqt_d.rearrange("(o p) m -> p o m", p=P))
            brow_sb = consts.tile([1, 3 * P], bf16)
            nc.sync.dma_start(brow_sb[:], brow_d[:])
            gq_sb = consts.tile([D, HC], f32)
            nc.sync.dma_start(gq_sb[:], gq_d[:])
            ind2_sb = consts.tile([P, VA], bf16)
            nc.sync.dma_start(ind2_sb[:], ind2_d[:])
            wk_sb = consts.tile([P, CC, P], bf16)
            nc.sync.dma_start(wk_sb[:], wkt_d.rearrange("(o p) m -> p o m", p=P))
            wv_sb = consts.tile([P, CC, P], bf16)
            nc.sync.dma_start(wv_sb[:], wvt_d.rearrange("(o p) m -> p o m", p=P))
            wo_sb = consts.tile([P, C], bf16)
            nc.sync.dma_start(wo_sb[:], wot_d[:])
            gk_sb = consts.tile([D, HC], f32)
            nc.sync.dma_start(gk_sb[:], gk_d[:])
            ident_sb = consts.tile([P, P], f32)
            nc.sync.dma_start(ident_sb[:], ident_d[:])
            mask_sb = consts.tile([P, B * MC], f32)
            nc.sync.dma_start(mask_sb[:], mask_d[:])
            eps_sb = consts.tile([VA, 1], f32)
            nc.vector.memset(eps_sb[:], EPS)
            ones_sb = consts.tile([1, 512], bf16)
            nc.vector.memset(ones_sb[:], 1.0)

            def proj(ps_dst, w_sb, src, brow):
                """ps_dst[128,1024] (psum) = w^T src + bias (seeded)."""
                for half in range(2):
                    hs = slice(half * 512, (half + 1) * 512)
                    nc.tensor.matmul(
                        ps_dst[:, hs], brow, ones_sb[:], start=True, stop=False
                    )
                    for cc in range(CC):
                        mv = src[cc][:, hs] if isinstance(src, list) else src[:, cc, hs]
                        nc.tensor.matmul(
                            ps_dst[:, hs],
                            w_sb[:, cc],
                            mv,
                            start=False,
                            stop=(cc == CC - 1),
                        )

            def rms_stage1(psrc):
                """Evacuate biased raw projections from PSUM (frees the
                bank) and square them.  Returns the in-flight state."""
                st = {}
                st["raw_a"] = work.tile([D, 1024], bf16, tag="raw_a", bufs=3, name="raw_a")
                nc.vector.tensor_copy(st["raw_a"][:], psrc[0:D, :])
                st["raw_b"] = work.tile([D, 1024], bf16, tag="raw_b", bufs=3, name="raw_b")
                nc.vector.tensor_copy(st["raw_b"][:], psrc[D:P, :])
                st["sq"] = work.tile([P, 1024], bf16, tag="sq", bufs=2, name="sq")
                nc.vector.tensor_mul(st["sq"][0:D, :], st["raw_a"][:], st["raw_a"][:])
                nc.vector.tensor_mul(st["sq"][D:P, :], st["raw_b"][:], st["raw_b"][:])
                return st

            def rms_stage2(st):
                """Per-head sum of squares -> rsqrt -> per-head broadcast."""
                ss = ps.tile([VA, 1024], f32, tag="O", bufs=2, name="ss")
                for half in range(2):
                    hs = slice(half * 512, (half + 1) * 512)
                    nc.tensor.matmul(
                        ss[:, hs], ind2_sb[:], st["sq"][:, hs], start=True, stop=True
                    )
                srt = work.tile([VA, 1024], f32, tag="srt", bufs=1, name="srt")
                nc.scalar.activation(
                    srt[:], ss[:], AF.Sqrt, scale=1.0 / D, bias=eps_sb[:]
                )
                rstd = work.tile([VA, 1024], f32, tag="rstd", bufs=2, name="rstd")
                nc.vector.reciprocal_approx_fast(out=rstd[:], in_=srt[:])
                r1 = work.tile([1, 1024], f32, tag="r1", bufs=2, name="r1")
                nc.vector.tensor_copy(r1[:], rstd[D : D + 1, :])
                st["rbc0"] = work.tile([D, 1024], f32, tag="rbc0", bufs=2, name="rbc0")
                nc.gpsimd.partition_broadcast(st["rbc0"][:], rstd[0:1, :])
                st["rbc1"] = work.tile([D, 1024], f32, tag="rbc1", bufs=2, name="rbc1")
                nc.gpsimd.partition_broadcast(st["rbc1"][:], r1[:])

            def rms_stage3(st):
                gcol, dst = st["gcol"], st["dst"]
                nc.vector.scalar_tensor_tensor(
                    out=dst[0:D, :], in0=st["raw_a"][:], scalar=gcol[:, 0:1],
                    in1=st["rbc0"][:], op0=MUL, op1=MUL,
                )
                nc.vector.scalar_tensor_tensor(
                    out=dst[D:P, :], in0=st["raw_b"][:], scalar=gcol[:, 1:2],
                    in1=st["rbc1"][:], op0=MUL, op1=MUL,
                )

            def rms_pump(q, force=False):
                """Software-pipeline the rms chains: stage2 runs one chunk
                behind stage1, stage3 two behind, so no engine queue ever
                waits head-of-line on a cross-engine dependency."""
                n = len(q)
                for idx, ent in enumerate(q):
                    age = n - idx
                    want = 3 if (age >= 3 or force) else (2 if age >= 2 else 1)
                    while ent["stage"] < want:
                        if ent["stage"] == 1:
                            rms_stage2(ent)
                        else:
                            rms_stage3(ent)
                        ent["stage"] += 1

            qtn = [
                qk.tile([P, 1024], bf16, tag=f"qtn{i}", name=f"qtn{i}")
                for i in range(B * NT)
            ]
            outp = qk.tile([P, N], bf16, tag="outtn")  # per-batch, reused

            def outproj_chunk(b, tc_):
                y_sb = work.tile([P, C], bf16, tag="ysb", bufs=3)
                ps_y = ps.tile([P, 1024], f32, tag="A", bufs=2)
                for ec in range(2):
                    nc.tensor.matmul(
                        ps_y[:, ec * 512 : (ec + 1) * 512],
                        outp[:, tc_ * P : (tc_ + 1) * P],
                        wo_sb[:, ec * 512 : (ec + 1) * 512],
                        start=True,
                        stop=True,
                    )
                if tc_ % 2 == 0:
                    nc.vector.tensor_copy(y_sb[:], ps_y[:])
                else:
                    nc.scalar.activation(y_sb[:], ps_y[:], AF.Copy)
                nc.sync.dma_start(y_d[b, tc_ * P : (tc_ + 1) * P, :], y_sb[:])

            def load_xt_chunk(xt_r, nt):
                xt_cc = []
                for cc in range(CC):
                    t = xp.tile([P, 1024], bf16, tag="xt")
                    nc.sync.dma_start(
                        t[:], xt_r[:, cc, nt * 1024 : (nt + 1) * 1024]
                    )
                    xt_cc.append(t)
                return xt_cc

            def load_ctx(b):
                t = ctxp.tile([P, CC, M], bf16, tag="ctx")
                nc.sync.dma_start(
                    t[:], ctxt_d[b].rearrange("(o p) m -> p o m", p=P)
                )
                return t

            prefetched = {}
            for b in range(B):
                xt_r = xt_d[b].rearrange("(o p) n -> p o n", p=P)
                pf = prefetched.pop(b, None)
                ctx_sb = pf["ctx"] if pf else None

                # ---- Q projection (pipelined rms) ----
                rq = []
                for nt in range(NT):
                    if pf is not None and nt == 0:
                        xt_cc = pf["xt0"]
                    else:
                        xt_cc = load_xt_chunk(xt_r, nt)
                    if ctx_sb is None:
                        ctx_sb = load_ctx(b)
                    ps_q = ps.tile([P, 1024], f32, tag="A", bufs=2)
                    proj(ps_q, wq_sb, xt_cc, brow_sb[:, 0:P])
                    st = rms_stage1(ps_q[:])
                    st.update(stage=1, gcol=gq_sb, dst=qtn[b * NT + nt][:])
                    rq.append(st)
                    rms_pump(rq)

                # ---- KV phase ----
                ktn = qk.tile([P, M], bf16, tag="ktn", bufs=2)
                ps_k = ps.tile([P, 1024], f32, tag="A", bufs=2)
                proj(ps_k, wk_sb, ctx_sb, brow_sb[:, P : 2 * P])
                st = rms_stage1(ps_k[:])
                st.update(stage=1, gcol=gk_sb, dst=ktn[:])
                rq.append(st)
                rms_pump(rq)

                ps_v = ps.tile([P, 1024], f32, tag="A", bufs=2)
                proj(ps_v, wv_sb, ctx_sb, brow_sb[:, 2 * P : 3 * P])
                rms_pump(rq, force=True)
                vt_sb = work.tile([P, M], f32, tag="vt", bufs=2)
                nc.vector.tensor_copy(vt_sb[:], ps_v[:])
                vaug = qk.tile([P, MC, 2 * VA], bf16, tag="vaug", bufs=2)
                for mc in range(MC):
                    ps_t = ps.tile([P, P], f32, tag="O", bufs=2)
                    nc.tensor.transpose(
                        ps_t[:], vt_sb[:, mc * P : (mc + 1) * P], ident_sb[:]
                    )
                    mcol = mask_sb[:, b * MC + mc : b * MC + mc + 1]
                    nc.vector.tensor_mul(
                        vaug[:, mc, 0:D], ps_t[:, 0:D], mcol.to_broadcast((P, D))
                    )
                    nc.vector.tensor_copy(vaug[:, mc, D : D + 1], mcol)
                    nc.vector.tensor_mul(
                        vaug[:, mc, VA : VA + D],
                        ps_t[:, D : 2 * D],
                        mcol.to_broadcast((P, D)),
                    )
                    nc.vector.tensor_copy(vaug[:, mc, VA + D : VA + D + 1], mcol)

                # ---- prefetch the next batch's inputs (rides the DMA
                # queues during this batch's attention phase) ----
                if b + 1 < B:
                    xt_r2 = xt_d[b + 1].rearrange("(o p) n -> p o n", p=P)
                    prefetched[b + 1] = {
                        "xt0": load_xt_chunk(xt_r2, 0),
                        "ctx": load_ctx(b + 1),
                    }

                # ---- attention ----
                for na in range(NA):
                    nsl = slice(na * 512, (na + 1) * 512)
                    qt = qtn[b * NT + na // 2]
                    qsl = slice((na % 2) * 512, (na % 2) * 512 + 512)
                    ps_o = ps.tile([VA, 1024], f32, tag="O", bufs=2, name="ps_o")
                    for mc in range(MC):
                        msl = slice(mc * P, (mc + 1) * P)
                        ps_s = ps.tile([P, 1024], f32, tag="A", bufs=2)
                        for h in range(2):
                            hsl = slice(D * h, D * (h + 1))
                            nc.tensor.matmul(
                                ps_s[:, h * 512 : (h + 1) * 512],
                                ktn[hsl, msl],
                                qt[hsl, qsl],
                                start=True,
                                stop=True,
                            )
                        pt = ptp.tile([P, 1024], bf16, tag="pt")
                        nc.scalar.activation(pt[:], ps_s[:], AF.Exp)
                        for h in range(2):
                            nc.tensor.matmul(
                                ps_o[:, h * 512 : (h + 1) * 512],
                                vaug[:, mc, h * VA : (h + 1) * VA],
                                pt[:, h * 512 : (h + 1) * 512],
                                start=(mc == 0),
                                stop=(mc == MC - 1),
                            )
                    # epilogue: divide by the denominator row (64) per head
                    den = work.tile([1, 1024], f32, tag="den", name="den", bufs=1)
                    nc.vector.tensor_copy(den[:], ps_o[D : D + 1, :])
                    rec = work.tile([1, 1024], f32, tag="rec", name="rec", bufs=1)
                    nc.vector.reciprocal_approx_fast(out=rec[:], in_=den[:])
                    rbc2 = work.tile([D, 1024], f32, tag="rbc2", name="rbc2", bufs=1)
                    nc.gpsimd.partition_broadcast(rbc2[:], rec[:])
                    nc.vector.scalar_tensor_tensor(
                        out=outp[0:D, nsl], in0=ps_o[0:D, 0:512], scalar=1.0,
                        in1=rbc2[:, 0:512], op0=MUL, op1=MUL,
                    )
                    nc.vector.scalar_tensor_tensor(
                        out=outp[D:P, nsl], in0=ps_o[0:D, 512:1024], scalar=1.0,
                        in1=rbc2[:, 512:1024], op0=MUL, op1=MUL,
                    )

                # ---- output projection phase (PE-dense) ----
                for tc_ in range(N // P):
                    outproj_chunk(b, tc_)

    nc.compile()
    _CACHE["nc"] = nc
    return nc


def _make_in_maps(x, context, context_mask, Wq, bq, Wkv, bkv, gq, gk, Wo, bo):
    import ml_dtypes

    f32 = np.float32
    bf16 = ml_dtypes.bfloat16
    xt = np.ascontiguousarray(np.transpose(x, (0, 2, 1))).astype(bf16)
    ctxt = np.ascontiguousarray(np.transpose(context, (0, 2, 1))).astype(bf16)
    # maskf[p, b*MC + mc] = mask[b, mc*128 + p]
    maskf = np.ascontiguousarray(
        np.transpose(
            np.asarray(context_mask, dtype=f32).reshape(B, MC, P), (2, 0, 1)
        ).reshape(P, B * MC)
    )
    ident = np.eye(P, dtype=f32)
    ind2 = np.zeros((P, VA), dtype=f32)
    ind2[0:D, 0] = 1.0
    ind2[D : 2 * D, D] = 1.0
    ind2 = ind2.astype(bf16)

    in_maps = []
    for c in range(8):
        hs = slice(P * c, P * (c + 1))
        gqcol = np.zeros((D, HC), dtype=f32)
        gkcol = np.zeros((D, HC), dtype=f32)
        for h in range(HC):
            gqcol[:, h] = gq[HC * c + h] * (1.0 / np.sqrt(D))
            gkcol[:, h] = gk[HC * c + h]
        brow = np.zeros((1, 3 * P), dtype=f32)
        brow[0, 0:P] = np.asarray(bq[hs], dtype=f32)
        brow[0, P : 2 * P] = np.asarray(bkv[hs], dtype=f32)
        brow[0, 2 * P : 3 * P] = np.asarray(
            bkv[C + P * c : C + P * (c + 1)], dtype=f32
        )
        in_maps.append(
            {
                "xt": xt,
                "ctxt": ctxt,
                "wqt": np.ascontiguousarray(Wq[hs].T).astype(bf16),
                "wkt": np.ascontiguousarray(Wkv[hs].T).astype(bf16),
                "wvt": np.ascontiguousarray(
                    Wkv[C + P * c : C + P * (c + 1)].T
                ).astype(bf16),
                "wot": np.ascontiguousarray(Wo[:, hs].T).astype(bf16),
                "brow": brow.astype(bf16),
                "gqcol": gqcol,
                "gkcol": gkcol,
                "ind2": ind2,
                "ident": ident,
                "maskf": maskf,
            }
        )
    return in_maps


def _run(in_maps, **spmd_kwargs):
    from concourse import bass_utils

    nc = _build()
    return bass_utils.run_bass_kernel_spmd(
        nc, in_maps, core_ids=list(range(8)), **spmd_kwargs
    )


def kernel(x, context, context_mask, Wq, bq, Wkv, bkv, gq, gk, Wo, bo):
    in_maps = _make_in_maps(
        x, context, context_mask, Wq, bq, Wkv, bkv, gq, gk, Wo, bo
    )
    res = _run(in_maps)
    y = np.zeros((B, N, C), dtype=np.float64)
    for c in range(8):
        y += np.asarray(res.results[c]["y"], dtype=np.float64)
    y += np.asarray(bo, dtype=np.float64)
    return y.astype(np.float32)


# revision 23
# speedup vs baseline: 1.0124x; 1.0041x over previous
"""Cross-attention kernel for Trainium2, sharded over 8 NeuronCores.

Problem (hardcoded shapes): B=2, N=4096, M=1024, DIM=1024, H=16, D=64.
  q = rms_norm(x @ Wq.T + bq)        per-head, gamma gq, eps 1e-6
  k = rms_norm(ctx @ Wk.T + bk)      (Wk = first half of Wkv)
  v = ctx @ Wv.T + bv                (Wv = second half of Wkv)
  out = softmax(q k^T / sqrt(D) + mask_bias) @ v
  y = out @ Wo.T + bo

Sharding: tensor-parallel over the 16 heads -> 2 heads per core.
Each core computes q/k/v projections for its 2 heads (column-sharded
Wq/Wkv), full attention for those heads, and a partial output
projection (row-sharded Wo).  The host sums the 8 partial outputs.

v3 design notes (vs the 609us baseline):
 - All matmul operands are bf16 (FWL halves LDWEIGHTS, DMA halves).
   PSUM stays f32; normalization math in f32.
 - Biases are seeded into PSUM with a [1,128] bias-row matmul against
   a ones vector, so no DVE pass is needed to add them.
 - rms_norm: per-head sum-of-squares via indicator matmul, sqrt on
   ACT, reciprocal on DVE, per-head broadcast on GPSIMD, and one
   scalar_tensor_tensor per head applies gamma*rstd straight out of
   PSUM.
 - Attention epilogue reciprocal reads PSUM directly; the out-proj
   matmuls of chunk nt are emitted after chunk nt+1's last PV matmul
   so the in-order PE queue always has ready work while the epilogue
   chain (DVE recip -> GPSIMD bcast -> DVE muls) completes.
 - Softmax denominator rides as a 65th column of V (ones, masked).
 - y is written as bf16 partials; the host sums the 8 cores.
"""

import numpy as np

P = 128
B = 2
N = 4096
M = 1024
C = 1024  # DIM == COND_DIM
H = 16
D = 64
HC = 2  # heads per core
CC = C // P  # contraction chunks
NT = N // 1024  # query chunks of 1024 (projection granularity)
NA = N // 512  # query chunks of 512 (attention granularity)
MC = M // P  # kv chunks of 128
VA = D + 1  # V columns + denominator column
EPS = 1e-6

_CACHE = {}


def _build():
    if "nc" in _CACHE:
        return _CACHE["nc"]

    import concourse.bass as bass  # noqa: F401
    import concourse.tile as tile
    from concourse import bacc, mybir

    f32 = mybir.dt.float32
    bf16 = mybir.dt.bfloat16
    AF = mybir.ActivationFunctionType
    MUL = mybir.AluOpType.mult

    nc = bacc.Bacc("TRN2", target_bir_lowering=False, debug=False, num_devices=8)

    xt_d = nc.dram_tensor("xt", [B, C, N], bf16, kind="ExternalInput").ap()
    ctxt_d = nc.dram_tensor("ctxt", [B, C, M], bf16, kind="ExternalInput").ap()
    wqt_d = nc.dram_tensor("wqt", [C, P], bf16, kind="ExternalInput").ap()
    wkt_d = nc.dram_tensor("wkt", [C, P], bf16, kind="ExternalInput").ap()
    wvt_d = nc.dram_tensor("wvt", [C, P], bf16, kind="ExternalInput").ap()
    wot_d = nc.dram_tensor("wot", [P, C], bf16, kind="ExternalInput").ap()
    brow_d = nc.dram_tensor("brow", [1, 3 * P], bf16, kind="ExternalInput").ap()
    gq_d = nc.dram_tensor("gqcol", [D, HC], f32, kind="ExternalInput").ap()
    gk_d = nc.dram_tensor("gkcol", [D, HC], f32, kind="ExternalInput").ap()
    ind2_d = nc.dram_tensor("ind2", [P, VA], bf16, kind="ExternalInput").ap()
    ident_d = nc.dram_tensor("ident", [P, P], f32, kind="ExternalInput").ap()
    mask_d = nc.dram_tensor("maskf", [P, B * MC], f32, kind="ExternalInput").ap()
    y_d = nc.dram_tensor("y", [B, N, C], bf16, kind="ExternalOutput").ap()

    with tile.TileContext(nc) as tc:
        with (
            tc.tile_pool(name="consts", bufs=1) as consts,
            tc.tile_pool(name="ctxp", bufs=2) as ctxp,
            tc.tile_pool(name="xp", bufs=16) as xp,
            tc.tile_pool(name="qk", bufs=1) as qk,
            tc.tile_pool(name="work", bufs=2) as work,
            tc.tile_pool(name="ptp", bufs=3) as ptp,
            tc.tile_pool(name="ps", bufs=1, space="PSUM") as ps,
        ):
            wq_sb = consts.tile([P, CC, P], bf16)
            nc.sync.dma_start(wq_sb[:], wqt_d.rearrange("(o p) m -> p o m", p=P))
            brow_sb = consts.tile([1, 3 * P], bf16)
            nc.sync.dma_start(brow_sb[:], brow_d[:])
            gq_sb = consts.tile([D, HC], f32)
            nc.sync.dma_start(gq_sb[:], gq_d[:])
            ind2_sb = consts.tile([P, VA], bf16)
            nc.sync.dma_start(ind2_sb[:], ind2_d[:])
            wk_sb = consts.tile([P, CC, P], bf16)
            nc.sync.dma_start(wk_sb[:], wkt_d.rearrange("(o p) m -> p o m", p=P))
            wv_sb = consts.tile([P, CC, P], bf16)
            nc.sync.dma_start(wv_sb[:], wvt_d.rearrange("(o p) m -> p o m", p=P))
            wo_sb = consts.tile([P, C], bf16)
            nc.sync.dma_start(wo_sb[:], wot_d[:])
            gk_sb = consts.tile([D, HC], f32)
            nc.sync.dma_start(gk_sb[:], gk_d[:])
            ident_sb = consts.tile([P, P], f32)
            nc.sync.dma_start(ident_sb[:], ident_d[:])
            mask_sb = consts.tile([P, B * MC], f32)
            nc.sync.dma_start(mask_sb[:], mask_d[:])
            eps_sb = consts.tile([VA, 1], f32)
            nc.vector.memset(eps_sb[:], EPS)
            ones_sb = consts.tile([1, 512], bf16)
            nc.vector.memset(ones_sb[:], 1.0)

            def proj(ps_dst, w_sb, src, brow):
                """ps_dst[128,1024] (psum) = w^T src + bias (seeded)."""
                for half in range(2):
                    hs = slice(half * 512, (half + 1) * 512)
                    nc.tensor.matmul(
                        ps_dst[:, hs], brow, ones_sb[:], start=True, stop=False
                    )
                    for cc in range(CC):
                        mv = src[cc][:, hs] if isinstance(src, list) else src[:, cc, hs]
                        nc.tensor.matmul(
                            ps_dst[:, hs],
                            w_sb[:, cc],
                            mv,
                            start=False,
                            stop=(cc == CC - 1),
                        )

            def rms_stage1(psrc):
                """Evacuate biased raw projections from PSUM (frees the
                bank) and square them.  Returns the in-flight state."""
                st = {}
                st["raw_a"] = work.tile([D, 1024], bf16, tag="raw_a", bufs=3, name="raw_a")
                nc.vector.tensor_copy(st["raw_a"][:], psrc[0:D, :])
                st["raw_b"] = work.tile([D, 1024], bf16, tag="raw_b", bufs=3, name="raw_b")
                nc.vector.tensor_copy(st["raw_b"][:], psrc[D:P, :])
                st["sq"] = work.tile([P, 1024], bf16, tag="sq", bufs=2, name="sq")
                nc.vector.tensor_mul(st["sq"][0:D, :], st["raw_a"][:], st["raw_a"][:])
                nc.vector.tensor_mul(st["sq"][D:P, :], st["raw_b"][:], st["raw_b"][:])
                return st

            def rms_stage2(st):
                """Per-head sum of squares -> rsqrt -> per-head broadcast."""
                ss = ps.tile([VA, 1024], f32, tag="O", bufs=2, name="ss")
                for half in range(2):
                    hs = slice(half * 512, (half + 1) * 512)
                    nc.tensor.matmul(
                        ss[:, hs], ind2_sb[:], st["sq"][:, hs], start=True, stop=True
                    )
                srt = work.tile([VA, 1024], f32, tag="srt", bufs=1, name="srt")
                nc.scalar.activation(
                    srt[:], ss[:], AF.Sqrt, scale=1.0 / D, bias=eps_sb[:]
                )
                rstd = work.tile([VA, 1024], f32, tag="rstd", bufs=2, name="rstd")
                nc.vector.reciprocal_approx_fast(out=rstd[:], in_=srt[:])
                r1 = work.tile([1, 1024], f32, tag="r1", bufs=2, name="r1")
                nc.vector.tensor_copy(r1[:], rstd[D : D + 1, :])
                st["rbc0"] = work.tile([D, 1024], f32, tag="rbc0", bufs=2, name="rbc0")
                nc.gpsimd.partition_broadcast(st["rbc0"][:], rstd[0:1, :])
                st["rbc1"] = work.tile([D, 1024], f32, tag="rbc1", bufs=2, name="rbc1")
                nc.gpsimd.partition_broadcast(st["rbc1"][:], r1[:])

            def rms_stage3(st):
                gcol, dst = st["gcol"], st["dst"]
                nc.vector.scalar_tensor_tensor(
                    out=dst[0:D, :], in0=st["raw_a"][:], scalar=gcol[:, 0:1],
                    in1=st["rbc0"][:], op0=MUL, op1=MUL,
                )
                nc.vector.scalar_tensor_tensor(
                    out=dst[D:P, :], in0=st["raw_b"][:], scalar=gcol[:, 1:2],
                    in1=st["rbc1"][:], op0=MUL, op1=MUL,
                )

            def rms_pump(q, force=False):
                """Software-pipeline the rms chains: stage2 runs one chunk
                behind stage1, stage3 two behind, so no engine queue ever
                waits head-of-line on a cross-engine dependency."""
                n = len(q)
                for idx, ent in enumerate(q):
                    age = n - idx
                    want = 3 if (age >= 3 or force) else (2 if age >= 2 else 1)
                    while ent["stage"] < want:
                        if ent["stage"] == 1:
                            rms_stage2(ent)
                        else:
                            rms_stage3(ent)
                        ent["stage"] += 1

            qtn = [
                qk.tile([P, 1024], bf16, tag=f"qtn{i}", name=f"qtn{i}")
                for i in range(B * NT)
            ]
            outp = qk.tile([P, N], bf16, tag="outtn")  # per-batch, reused

            def outproj_chunk(b, tc_):
                y_sb = work.tile([P, C], bf16, tag="ysb", bufs=3)
                ps_y = ps.tile([P, 1024], f32, tag="A", bufs=2)
                for ec in range(2):
                    nc.tensor.matmul(
                        ps_y[:, ec * 512 : (ec + 1) * 512],
                        outp[:, tc_ * P : (tc_ + 1) * P],
                        wo_sb[:, ec * 512 : (ec + 1) * 512],
                        start=True,
                        stop=True,
                    )
                if tc_ % 2 == 0:
                    nc.vector.tensor_copy(y_sb[:], ps_y[:])
                else:
                    nc.scalar.activation(y_sb[:], ps_y[:], AF.Copy)
                nc.scalar.dma_start(y_d[b, tc_ * P : (tc_ + 1) * P, :], y_sb[:])

            def load_xt_chunk(xt_r, nt):
                xt_cc = []
                for cc in range(CC):
                    t = xp.tile([P, 1024], bf16, tag="xt")
                    nc.sync.dma_start(
                        t[:], xt_r[:, cc, nt * 1024 : (nt + 1) * 1024]
                    )
                    xt_cc.append(t)
                return xt_cc

            def load_ctx(b):
                t = ctxp.tile([P, CC, M], bf16, tag="ctx")
                nc.sync.dma_start(
                    t[:], ctxt_d[b].rearrange("(o p) m -> p o m", p=P)
                )
                return t

            prefetched = {}
            for b in range(B):
                xt_r = xt_d[b].rearrange("(o p) n -> p o n", p=P)
                pf = prefetched.pop(b, None)
                ctx_sb = pf["ctx"] if pf else None

                # ---- Q projection chunk 0 (the rest interleave with
                # attention so their xt DMAs ride the attention window) ----
                rq = []
                if pf is not None:
                    xt_cc = pf["xt0"]
                else:
                    xt_cc = load_xt_chunk(xt_r, 0)
                if ctx_sb is None:
                    ctx_sb = load_ctx(b)
                ps_q = ps.tile([P, 1024], f32, tag="A", bufs=2)
                proj(ps_q, wq_sb, xt_cc, brow_sb[:, 0:P])
                st = rms_stage1(ps_q[:])
                st.update(stage=1, gcol=gq_sb, dst=qtn[b * NT][:])
                rq.append(st)
                rms_pump(rq)

                # ---- KV phase ----
                ktn = qk.tile([P, M], bf16, tag="ktn", bufs=2)
                ps_k = ps.tile([P, 1024], f32, tag="A", bufs=2)
                proj(ps_k, wk_sb, ctx_sb, brow_sb[:, P : 2 * P])
                st = rms_stage1(ps_k[:])
                st.update(stage=1, gcol=gk_sb, dst=ktn[:])
                rq.append(st)
                rms_pump(rq)

                ps_v = ps.tile([P, 1024], f32, tag="A", bufs=2)
                proj(ps_v, wv_sb, ctx_sb, brow_sb[:, 2 * P : 3 * P])
                rms_pump(rq, force=True)
                vt_sb = work.tile([P, M], f32, tag="vt", bufs=2)
                nc.vector.tensor_copy(vt_sb[:], ps_v[:])
                vaug = qk.tile([P, MC, 2 * VA], bf16, tag="vaug", bufs=2)
                for mc in range(MC):
                    ps_t = ps.tile([P, P], f32, tag="O", bufs=2)
                    nc.tensor.transpose(
                        ps_t[:], vt_sb[:, mc * P : (mc + 1) * P], ident_sb[:]
                    )
                    mcol = mask_sb[:, b * MC + mc : b * MC + mc + 1]
                    nc.vector.tensor_mul(
                        vaug[:, mc, 0:D], ps_t[:, 0:D], mcol.to_broadcast((P, D))
                    )
                    nc.vector.tensor_copy(vaug[:, mc, D : D + 1], mcol)
                    nc.vector.tensor_mul(
                        vaug[:, mc, VA : VA + D],
                        ps_t[:, D : 2 * D],
                        mcol.to_broadcast((P, D)),
                    )
                    nc.vector.tensor_copy(vaug[:, mc, VA + D : VA + D + 1], mcol)

                # ---- prefetch the next batch's inputs (rides the DMA
                # queues during this batch's attention phase) ----
                if b + 1 < B:
                    xt_r2 = xt_d[b + 1].rearrange("(o p) n -> p o n", p=P)
                    prefetched[b + 1] = {
                        "xt0": load_xt_chunk(xt_r2, 0),
                        "ctx": load_ctx(b + 1),
                    }

                # ---- attention (with interleaved Q projections) ----
                pending_xt = {}
                for na in range(NA):
                    if na in (0, 2, 4):
                        pending_xt[na // 2 + 1] = load_xt_chunk(xt_r, na // 2 + 1)
                    if na in (1, 3, 5):
                        nt = (na + 1) // 2
                        ps_q2 = ps.tile([P, 1024], f32, tag="A", bufs=2)
                        proj(ps_q2, wq_sb, pending_xt.pop(nt), brow_sb[:, 0:P])
                        st2 = rms_stage1(ps_q2[:])
                        st2.update(stage=1, gcol=gq_sb, dst=qtn[b * NT + nt][:])
                        rms_stage2(st2)
                        rms_stage3(st2)
                    nsl = slice(na * 512, (na + 1) * 512)
                    qt = qtn[b * NT + na // 2]
                    qsl = slice((na % 2) * 512, (na % 2) * 512 + 512)
                    ps_o = ps.tile([VA, 1024], f32, tag="O", bufs=2, name="ps_o")
                    for mc in range(MC):
                        msl = slice(mc * P, (mc + 1) * P)
                        ps_s = ps.tile([P, 1024], f32, tag="A", bufs=2)
                        for h in range(2):
                            hsl = slice(D * h, D * (h + 1))
                            nc.tensor.matmul(
                                ps_s[:, h * 512 : (h + 1) * 512],
                                ktn[hsl, msl],
                                qt[hsl, qsl],
                                start=True,
                                stop=True,
                            )
                        pt = ptp.tile([P, 1024], bf16, tag="pt")
                        nc.scalar.activation(pt[:], ps_s[:], AF.Exp)
                        for h in range(2):
                            nc.tensor.matmul(
                                ps_o[:, h * 512 : (h + 1) * 512],
                                vaug[:, mc, h * VA : (h + 1) * VA],
                                pt[:, h * 512 : (h + 1) * 512],
                                start=(mc == 0),
                                stop=(mc == MC - 1),
                            )
                    # epilogue: divide by the denominator row (64) per head
                    den = work.tile([1, 1024], f32, tag="den", name="den", bufs=1)
                    nc.vector.tensor_copy(den[:], ps_o[D : D + 1, :])
                    rec = work.tile([1, 1024], f32, tag="rec", name="rec", bufs=1)
                    nc.vector.reciprocal_approx_fast(out=rec[:], in_=den[:])
                    rbc2 = work.tile([D, 1024], f32, tag="rbc2", name="rbc2", bufs=1)
                    nc.gpsimd.partition_broadcast(rbc2[:], rec[:])
                    nc.vector.scalar_tensor_tensor(
                        out=outp[0:D, nsl], in0=ps_o[0:D, 0:512], scalar=1.0,
                        in1=rbc2[:, 0:512], op0=MUL, op1=MUL,
                    )
                    nc.vector.scalar_tensor_tensor(
                        out=outp[D:P, nsl], in0=ps_o[0:D, 512:1024], scalar=1.0,
                        in1=rbc2[:, 512:1024], op0=MUL, op1=MUL,
                    )

                # ---- output projection phase (PE-dense) ----
                for tc_ in range(N // P):
                    outproj_chunk(b, tc_)

    nc.compile()
    _CACHE["nc"] = nc
    return nc


def _make_in_maps(x, context, context_mask, Wq, bq, Wkv, bkv, gq, gk, Wo, bo):
    import ml_dtypes

    f32 = np.float32
    bf16 = ml_dtypes.bfloat16
    xt = np.ascontiguousarray(np.transpose(x, (0, 2, 1))).astype(bf16)
    ctxt = np.ascontiguousarray(np.transpose(context, (0, 2, 1))).astype(bf16)
    # maskf[p, b*MC + mc] = mask[b, mc*128 + p]
    maskf = np.ascontiguousarray(
        np.transpose(
            np.asarray(context_mask, dtype=f32).reshape(B, MC, P), (2, 0, 1)
        ).reshape(P, B * MC)
    )
    ident = np.eye(P, dtype=f32)
    ind2 = np.zeros((P, VA), dtype=f32)
    ind2[0:D, 0] = 1.0
    ind2[D : 2 * D, D] = 1.0
    ind2 = ind2.astype(bf16)

    in_maps = []
    for c in range(8):
        hs = slice(P * c, P * (c + 1))
        gqcol = np.zeros((D, HC), dtype=f32)
        gkcol = np.zeros((D, HC), dtype=f32)
        for h in range(HC):
            gqcol[:, h] = gq[HC * c + h] * (1.0 / np.sqrt(D))
            gkcol[:, h] = gk[HC * c + h]
        brow = np.zeros((1, 3 * P), dtype=f32)
        brow[0, 0:P] = np.asarray(bq[hs], dtype=f32)
        brow[0, P : 2 * P] = np.asarray(bkv[hs], dtype=f32)
        brow[0, 2 * P : 3 * P] = np.asarray(
            bkv[C + P * c : C + P * (c + 1)], dtype=f32
        )
        in_maps.append(
            {
                "xt": xt,
                "ctxt": ctxt,
                "wqt": np.ascontiguousarray(Wq[hs].T).astype(bf16),
                "wkt": np.ascontiguousarray(Wkv[hs].T).astype(bf16),
                "wvt": np.ascontiguousarray(
                    Wkv[C + P * c : C + P * (c + 1)].T
                ).astype(bf16),
                "wot": np.ascontiguousarray(Wo[:, hs].T).astype(bf16),
                "brow": brow.astype(bf16),
                "gqcol": gqcol,
                "gkcol": gkcol,
                "ind2": ind2,
                "ident": ident,
                "maskf": maskf,
            }
        )
    return in_maps


def _run(in_maps, **spmd_kwargs):
    from concourse import bass_utils

    nc = _build()
    return bass_utils.run_bass_kernel_spmd(
        nc, in_maps, core_ids=list(range(8)), **spmd_kwargs
    )


def kernel(x, context, context_mask, Wq, bq, Wkv, bkv, gq, gk, Wo, bo):
    in_maps = _make_in_maps(
        x, context, context_mask, Wq, bq, Wkv, bkv, gq, gk, Wo, bo
    )
    res = _run(in_maps)
    y = np.zeros((B, N, C), dtype=np.float64)
    for c in range(8):
        y += np.asarray(res.results[c]["y"], dtype=np.float64)
    y += np.asarray(bo, dtype=np.float64)
    return y.astype(np.float32)


# revision 24
# speedup vs baseline: 1.0470x; 1.0342x over previous
"""Cross-attention kernel for Trainium2, sharded over 8 NeuronCores.

Problem (hardcoded shapes): B=2, N=4096, M=1024, DIM=1024, H=16, D=64.
  q = rms_norm(x @ Wq.T + bq)        per-head, gamma gq, eps 1e-6
  k = rms_norm(ctx @ Wk.T + bk)      (Wk = first half of Wkv)
  v = ctx @ Wv.T + bv                (Wv = second half of Wkv)
  out = softmax(q k^T / sqrt(D) + mask_bias) @ v
  y = out @ Wo.T + bo

Sharding: tensor-parallel over the 16 heads -> 2 heads per core.
Each core computes q/k/v projections for its 2 heads (column-sharded
Wq/Wkv), full attention for those heads, and a partial output
projection (row-sharded Wo).  The host sums the 8 partial outputs.

v3 design notes (vs the 609us baseline):
 - All matmul operands are bf16 (FWL halves LDWEIGHTS, DMA halves).
   PSUM stays f32; normalization math in f32.
 - Biases are seeded into PSUM with a [1,128] bias-row matmul against
   a ones vector, so no DVE pass is needed to add them.
 - rms_norm: per-head sum-of-squares via indicator matmul, sqrt on
   ACT, reciprocal on DVE, per-head broadcast on GPSIMD, and one
   scalar_tensor_tensor per head applies gamma*rstd straight out of
   PSUM.
 - Attention epilogue reciprocal reads PSUM directly; the out-proj
   matmuls of chunk nt are emitted after chunk nt+1's last PV matmul
   so the in-order PE queue always has ready work while the epilogue
   chain (DVE recip -> GPSIMD bcast -> DVE muls) completes.
 - Softmax denominator rides as a 65th column of V (ones, masked).
 - y is written as bf16 partials; the host sums the 8 cores.
"""

import numpy as np

P = 128
B = 2
N = 4096
M = 1024
C = 1024  # DIM == COND_DIM
H = 16
D = 64
HC = 2  # heads per core
CC = C // P  # contraction chunks
NT = N // 1024  # query chunks of 1024 (projection granularity)
NA = N // 512  # query chunks of 512 (attention granularity)
MC = M // P  # kv chunks of 128
VA = D + 1  # V columns + denominator column
EPS = 1e-6

_CACHE = {}


def _build():
    if "nc" in _CACHE:
        return _CACHE["nc"]

    import concourse.bass as bass  # noqa: F401
    import concourse.tile as tile
    from concourse import bacc, mybir

    f32 = mybir.dt.float32
    bf16 = mybir.dt.bfloat16
    AF = mybir.ActivationFunctionType
    MUL = mybir.AluOpType.mult

    nc = bacc.Bacc("TRN2", target_bir_lowering=False, debug=False, num_devices=8)

    xt_d = nc.dram_tensor("xt", [B, C, N], bf16, kind="ExternalInput").ap()
    ctxt_d = nc.dram_tensor("ctxt", [B, C, M], bf16, kind="ExternalInput").ap()
    wqt_d = nc.dram_tensor("wqt", [C, P], bf16, kind="ExternalInput").ap()
    wkt_d = nc.dram_tensor("wkt", [C, P], bf16, kind="ExternalInput").ap()
    wvt_d = nc.dram_tensor("wvt", [C, P], bf16, kind="ExternalInput").ap()
    wot_d = nc.dram_tensor("wot", [P, C], bf16, kind="ExternalInput").ap()
    brow_d = nc.dram_tensor("brow", [1, 3 * P], bf16, kind="ExternalInput").ap()
    gq_d = nc.dram_tensor("gqcol", [D, HC], f32, kind="ExternalInput").ap()
    gk_d = nc.dram_tensor("gkcol", [D, HC], f32, kind="ExternalInput").ap()
    ind2_d = nc.dram_tensor("ind2", [P, VA], bf16, kind="ExternalInput").ap()
    ident_d = nc.dram_tensor("ident", [P, P], f32, kind="ExternalInput").ap()
    mask_d = nc.dram_tensor("maskf", [P, B * MC], f32, kind="ExternalInput").ap()
    y_d = nc.dram_tensor("y", [B, N, C], bf16, kind="ExternalOutput").ap()

    with tile.TileContext(nc) as tc:
        with (
            tc.tile_pool(name="consts", bufs=1) as consts,
            tc.tile_pool(name="ctxp", bufs=2) as ctxp,
            tc.tile_pool(name="xp", bufs=16) as xp,
            tc.tile_pool(name="qk", bufs=1) as qk,
            tc.tile_pool(name="work", bufs=2) as work,
            tc.tile_pool(name="ptp", bufs=3) as ptp,
            tc.tile_pool(name="ps", bufs=1, space="PSUM") as ps,
        ):
            wq_sb = consts.tile([P, CC, P], bf16)
            nc.sync.dma_start(wq_sb[:], wqt_d.rearrange("(o p) m -> p o m", p=P))
            brow_sb = consts.tile([1, 3 * P], bf16)
            nc.sync.dma_start(brow_sb[:], brow_d[:])
            gq_sb = consts.tile([D, HC], f32)
            nc.sync.dma_start(gq_sb[:], gq_d[:])
            ind2_sb = consts.tile([P, VA], bf16)
            nc.sync.dma_start(ind2_sb[:], ind2_d[:])
            wk_sb = consts.tile([P, CC, P], bf16)
            nc.sync.dma_start(wk_sb[:], wkt_d.rearrange("(o p) m -> p o m", p=P))
            wv_sb = consts.tile([P, CC, P], bf16)
            nc.sync.dma_start(wv_sb[:], wvt_d.rearrange("(o p) m -> p o m", p=P))
            wo_sb = consts.tile([P, C], bf16)
            nc.sync.dma_start(wo_sb[:], wot_d[:])
            gk_sb = consts.tile([D, HC], f32)
            nc.sync.dma_start(gk_sb[:], gk_d[:])
            ident_sb = consts.tile([P, P], f32)
            nc.sync.dma_start(ident_sb[:], ident_d[:])
            mask_sb = consts.tile([P, B * MC], f32)
            nc.sync.dma_start(mask_sb[:], mask_d[:])
            eps_sb = consts.tile([VA, 1], f32)
            nc.vector.memset(eps_sb[:], EPS)
            ones_sb = consts.tile([1, 512], bf16)
            nc.vector.memset(ones_sb[:], 1.0)

            def proj(ps_dst, w_sb, src, brow):
                """ps_dst[128,1024] (psum) = w^T src + bias (seeded)."""
                for half in range(2):
                    hs = slice(half * 512, (half + 1) * 512)
                    nc.tensor.matmul(
                        ps_dst[:, hs], brow, ones_sb[:], start=True, stop=False
                    )
                    for cc in range(CC):
                        mv = src[cc][:, hs] if isinstance(src, list) else src[:, cc, hs]
                        nc.tensor.matmul(
                            ps_dst[:, hs],
                            w_sb[:, cc],
                            mv,
                            start=False,
                            stop=(cc == CC - 1),
                        )

            def rms_stage1(psrc):
                """Evacuate biased raw projections from PSUM (frees the
                bank) and square them.  Returns the in-flight state."""
                st = {}
                st["raw_a"] = work.tile([D, 1024], bf16, tag="raw_a", bufs=3, name="raw_a")
                nc.vector.tensor_copy(st["raw_a"][:], psrc[0:D, :])
                st["raw_b"] = work.tile([D, 1024], bf16, tag="raw_b", bufs=3, name="raw_b")
                nc.vector.tensor_copy(st["raw_b"][:], psrc[D:P, :])
                st["sq"] = work.tile([P, 1024], bf16, tag="sq", bufs=2, name="sq")
                nc.vector.tensor_mul(st["sq"][0:D, :], st["raw_a"][:], st["raw_a"][:])
                nc.vector.tensor_mul(st["sq"][D:P, :], st["raw_b"][:], st["raw_b"][:])
                return st

            def rms_stage2(st):
                """Per-head sum of squares -> rsqrt -> per-head broadcast."""
                ss = ps.tile([VA, 1024], f32, tag="O", bufs=2, name="ss")
                for half in range(2):
                    hs = slice(half * 512, (half + 1) * 512)
                    nc.tensor.matmul(
                        ss[:, hs], ind2_sb[:], st["sq"][:, hs], start=True, stop=True
                    )
                srt = work.tile([VA, 1024], f32, tag="srt", bufs=1, name="srt")
                nc.scalar.activation(
                    srt[:], ss[:], AF.Sqrt, scale=1.0 / D, bias=eps_sb[:]
                )
                rstd = work.tile([VA, 1024], f32, tag="rstd", bufs=2, name="rstd")
                nc.vector.reciprocal_approx_fast(out=rstd[:], in_=srt[:])
                r1 = work.tile([1, 1024], f32, tag="r1", bufs=2, name="r1")
                nc.vector.tensor_copy(r1[:], rstd[D : D + 1, :])
                st["rbc0"] = work.tile([D, 1024], f32, tag="rbc0", bufs=2, name="rbc0")
                nc.gpsimd.partition_broadcast(st["rbc0"][:], rstd[0:1, :])
                st["rbc1"] = work.tile([D, 1024], f32, tag="rbc1", bufs=2, name="rbc1")
                nc.gpsimd.partition_broadcast(st["rbc1"][:], r1[:])

            def rms_stage3(st):
                gcol, dst = st["gcol"], st["dst"]
                nc.vector.scalar_tensor_tensor(
                    out=dst[0:D, :], in0=st["raw_a"][:], scalar=gcol[:, 0:1],
                    in1=st["rbc0"][:], op0=MUL, op1=MUL,
                )
                nc.vector.scalar_tensor_tensor(
                    out=dst[D:P, :], in0=st["raw_b"][:], scalar=gcol[:, 1:2],
                    in1=st["rbc1"][:], op0=MUL, op1=MUL,
                )

            def rms_pump(q, force=False):
                """Software-pipeline the rms chains: stage2 runs one chunk
                behind stage1, stage3 two behind, so no engine queue ever
                waits head-of-line on a cross-engine dependency."""
                n = len(q)
                for idx, ent in enumerate(q):
                    age = n - idx
                    want = 3 if (age >= 3 or force) else (2 if age >= 2 else 1)
                    while ent["stage"] < want:
                        if ent["stage"] == 1:
                            rms_stage2(ent)
                        else:
                            rms_stage3(ent)
                        ent["stage"] += 1

            qtn = [
                qk.tile([P, 1024], bf16, tag=f"qtn{i}", name=f"qtn{i}")
                for i in range(B * NT)
            ]
            outp = qk.tile([P, N], bf16, tag="outtn")  # per-batch, reused

            def outproj_chunk(b, tc_):
                y_sb = work.tile([P, C], bf16, tag="ysb", bufs=3)
                ps_y = ps.tile([P, 1024], f32, tag="A", bufs=2)
                for ec in range(2):
                    nc.tensor.matmul(
                        ps_y[:, ec * 512 : (ec + 1) * 512],
                        outp[:, tc_ * P : (tc_ + 1) * P],
                        wo_sb[:, ec * 512 : (ec + 1) * 512],
                        start=True,
                        stop=True,
                    )
                if tc_ % 2 == 0:
                    nc.vector.tensor_copy(y_sb[:], ps_y[:])
                else:
                    nc.scalar.activation(y_sb[:], ps_y[:], AF.Copy)
                nc.scalar.dma_start(y_d[b, tc_ * P : (tc_ + 1) * P, :], y_sb[:])

            def load_xt_chunk(xt_r, nt):
                xt_cc = []
                for cc in range(CC):
                    t = xp.tile([P, 1024], bf16, tag="xt")
                    nc.sync.dma_start(
                        t[:], xt_r[:, cc, nt * 1024 : (nt + 1) * 1024]
                    )
                    xt_cc.append(t)
                return xt_cc

            def load_ctx(b):
                t = ctxp.tile([P, CC, M], bf16, tag="ctx")
                nc.sync.dma_start(
                    t[:], ctxt_d[b].rearrange("(o p) m -> p o m", p=P)
                )
                return t

            prefetched = {}
            outproj_todo = []
            for b in range(B):
                xt_r = xt_d[b].rearrange("(o p) n -> p o n", p=P)
                pf = prefetched.pop(b, None)
                ctx_sb = pf["ctx"] if pf else None

                def drain_outproj(k):
                    for _ in range(min(k, len(outproj_todo))):
                        outproj_chunk(*outproj_todo.pop(0))

                # ---- prologue: Q/K/V projections (pipelined rms),
                # interleaved with the previous batch's out-projection ----
                rq = []
                for nt in range(NT):
                    if pf is not None:
                        xt_cc = pf["xt"][nt]
                    else:
                        xt_cc = load_xt_chunk(xt_r, nt)
                    if ctx_sb is None:
                        ctx_sb = load_ctx(b)
                    ps_q = ps.tile([P, 1024], f32, tag="A", bufs=2)
                    proj(ps_q, wq_sb, xt_cc, brow_sb[:, 0:P])
                    st = rms_stage1(ps_q[:])
                    st.update(stage=1, gcol=gq_sb, dst=qtn[b * NT + nt][:])
                    rq.append(st)
                    rms_pump(rq)
                    drain_outproj(6)

                ktn = qk.tile([P, M], bf16, tag="ktn", bufs=2)
                ps_k = ps.tile([P, 1024], f32, tag="A", bufs=2)
                proj(ps_k, wk_sb, ctx_sb, brow_sb[:, P : 2 * P])
                st = rms_stage1(ps_k[:])
                st.update(stage=1, gcol=gk_sb, dst=ktn[:])
                rq.append(st)
                rms_pump(rq)
                drain_outproj(4)

                ps_v = ps.tile([P, 1024], f32, tag="A", bufs=2)
                proj(ps_v, wv_sb, ctx_sb, brow_sb[:, 2 * P : 3 * P])
                rms_pump(rq, force=True)
                vt_sb = work.tile([P, M], f32, tag="vt", bufs=2)
                nc.vector.tensor_copy(vt_sb[:], ps_v[:])
                drain_outproj(4)
                vaug = qk.tile([P, MC, 2 * VA], bf16, tag="vaug", bufs=2)
                for mc in range(MC):
                    ps_t = ps.tile([P, P], f32, tag="O", bufs=2)
                    nc.tensor.transpose(
                        ps_t[:], vt_sb[:, mc * P : (mc + 1) * P], ident_sb[:]
                    )
                    mcol = mask_sb[:, b * MC + mc : b * MC + mc + 1]
                    nc.vector.tensor_mul(
                        vaug[:, mc, 0:D], ps_t[:, 0:D], mcol.to_broadcast((P, D))
                    )
                    nc.vector.tensor_copy(vaug[:, mc, D : D + 1], mcol)
                    nc.vector.tensor_mul(
                        vaug[:, mc, VA : VA + D],
                        ps_t[:, D : 2 * D],
                        mcol.to_broadcast((P, D)),
                    )
                    nc.vector.tensor_copy(vaug[:, mc, VA + D : VA + D + 1], mcol)
                drain_outproj(len(outproj_todo))

                # ---- prefetch the next batch's inputs (rides the DMA
                # queues during this batch's attention phase) ----
                if b + 1 < B:
                    xt_r2 = xt_d[b + 1].rearrange("(o p) n -> p o n", p=P)
                    prefetched[b + 1] = {
                        "xt": [load_xt_chunk(xt_r2, nt) for nt in range(NT)],
                        "ctx": load_ctx(b + 1),
                    }

                # ---- attention ----
                for na in range(NA):
                    nsl = slice(na * 512, (na + 1) * 512)
                    qt = qtn[b * NT + na // 2]
                    qsl = slice((na % 2) * 512, (na % 2) * 512 + 512)
                    ps_o = ps.tile([VA, 1024], f32, tag="O", bufs=2, name="ps_o")
                    for mc in range(MC):
                        msl = slice(mc * P, (mc + 1) * P)
                        ps_s = ps.tile([P, 1024], f32, tag="A", bufs=2)
                        for h in range(2):
                            hsl = slice(D * h, D * (h + 1))
                            nc.tensor.matmul(
                                ps_s[:, h * 512 : (h + 1) * 512],
                                ktn[hsl, msl],
                                qt[hsl, qsl],
                                start=True,
                                stop=True,
                            )
                        pt = ptp.tile([P, 1024], bf16, tag="pt")
                        nc.scalar.activation(pt[:], ps_s[:], AF.Exp)
                        for h in range(2):
                            nc.tensor.matmul(
                                ps_o[:, h * 512 : (h + 1) * 512],
                                vaug[:, mc, h * VA : (h + 1) * VA],
                                pt[:, h * 512 : (h + 1) * 512],
                                start=(mc == 0),
                                stop=(mc == MC - 1),
                            )
                    # epilogue: divide by the denominator row (64) per head
                    den = work.tile([1, 1024], f32, tag="den", name="den", bufs=1)
                    nc.vector.tensor_copy(den[:], ps_o[D : D + 1, :])
                    rec = work.tile([1, 1024], f32, tag="rec", name="rec", bufs=1)
                    nc.vector.reciprocal_approx_fast(out=rec[:], in_=den[:])
                    rbc2 = work.tile([D, 1024], f32, tag="rbc2", name="rbc2", bufs=1)
                    nc.gpsimd.partition_broadcast(rbc2[:], rec[:])
                    nc.vector.scalar_tensor_tensor(
                        out=outp[0:D, nsl], in0=ps_o[0:D, 0:512], scalar=1.0,
                        in1=rbc2[:, 0:512], op0=MUL, op1=MUL,
                    )
                    nc.vector.scalar_tensor_tensor(
                        out=outp[D:P, nsl], in0=ps_o[0:D, 512:1024], scalar=1.0,
                        in1=rbc2[:, 512:1024], op0=MUL, op1=MUL,
                    )

                # ---- queue this batch's out-projection; the last batch
                # drains it immediately (tail) ----
                outproj_todo = [(b, tc_) for tc_ in range(N // P)]
                if b == B - 1:
                    for ent in outproj_todo:
                        outproj_chunk(*ent)
                    outproj_todo = []

    nc.compile()
    _CACHE["nc"] = nc
    return nc


def _make_in_maps(x, context, context_mask, Wq, bq, Wkv, bkv, gq, gk, Wo, bo):
    import ml_dtypes

    f32 = np.float32
    bf16 = ml_dtypes.bfloat16
    xt = np.ascontiguousarray(np.transpose(x, (0, 2, 1))).astype(bf16)
    ctxt = np.ascontiguousarray(np.transpose(context, (0, 2, 1))).astype(bf16)
    # maskf[p, b*MC + mc] = mask[b, mc*128 + p]
    maskf = np.ascontiguousarray(
        np.transpose(
            np.asarray(context_mask, dtype=f32).reshape(B, MC, P), (2, 0, 1)
        ).reshape(P, B * MC)
    )
    ident = np.eye(P, dtype=f32)
    ind2 = np.zeros((P, VA), dtype=f32)
    ind2[0:D, 0] = 1.0
    ind2[D : 2 * D, D] = 1.0
    ind2 = ind2.astype(bf16)

    in_maps = []
    for c in range(8):
        hs = slice(P * c, P * (c + 1))
        gqcol = np.zeros((D, HC), dtype=f32)
        gkcol = np.zeros((D, HC), dtype=f32)
        for h in range(HC):
            gqcol[:, h] = gq[HC * c + h] * (1.0 / np.sqrt(D))
            gkcol[:, h] = gk[HC * c + h]
        brow = np.zeros((1, 3 * P), dtype=f32)
        brow[0, 0:P] = np.asarray(bq[hs], dtype=f32)
        brow[0, P : 2 * P] = np.asarray(bkv[hs], dtype=f32)
        brow[0, 2 * P : 3 * P] = np.asarray(
            bkv[C + P * c : C + P * (c + 1)], dtype=f32
        )
        in_maps.append(
            {
                "xt": xt,
                "ctxt": ctxt,
                "wqt": np.ascontiguousarray(Wq[hs].T).astype(bf16),
                "wkt": np.ascontiguousarray(Wkv[hs].T).astype(bf16),
                "wvt": np.ascontiguousarray(
                    Wkv[C + P * c : C + P * (c + 1)].T
                ).astype(bf16),
                "wot": np.ascontiguousarray(Wo[:, hs].T).astype(bf16),
                "brow": brow.astype(bf16),
                "gqcol": gqcol,
                "gkcol": gkcol,
                "ind2": ind2,
                "ident": ident,
                "maskf": maskf,
            }
        )
    return in_maps


def _run(in_maps, **spmd_kwargs):
    from concourse import bass_utils

    nc = _build()
    return bass_utils.run_bass_kernel_spmd(
        nc, in_maps, core_ids=list(range(8)), **spmd_kwargs
    )


def kernel(x, context, context_mask, Wq, bq, Wkv, bkv, gq, gk, Wo, bo):
    in_maps = _make_in_maps(
        x, context, context_mask, Wq, bq, Wkv, bkv, gq, gk, Wo, bo
    )
    res = _run(in_maps)
    y = np.zeros((B, N, C), dtype=np.float64)
    for c in range(8):
        y += np.asarray(res.results[c]["y"], dtype=np.float64)
    y += np.asarray(bo, dtype=np.float64)
    return y.astype(np.float32)


# revision 26
# speedup vs baseline: 1.0673x; 1.0194x over previous
"""Cross-attention kernel for Trainium2, sharded over 8 NeuronCores.

Problem (hardcoded shapes): B=2, N=4096, M=1024, DIM=1024, H=16, D=64.
  q = rms_norm(x @ Wq.T + bq)        per-head, gamma gq, eps 1e-6
  k = rms_norm(ctx @ Wk.T + bk)      (Wk = first half of Wkv)
  v = ctx @ Wv.T + bv                (Wv = second half of Wkv)
  out = softmax(q k^T / sqrt(D) + mask_bias) @ v
  y = out @ Wo.T + bo

Sharding: tensor-parallel over the 16 heads -> 2 heads per core.
Each core computes q/k/v projections for its 2 heads (column-sharded
Wq/Wkv), full attention for those heads, and a partial output
projection (row-sharded Wo).  The host sums the 8 partial outputs.

v3 design notes (vs the 609us baseline):
 - All matmul operands are bf16 (FWL halves LDWEIGHTS, DMA halves).
   PSUM stays f32; normalization math in f32.
 - Biases are seeded into PSUM with a [1,128] bias-row matmul against
   a ones vector, so no DVE pass is needed to add them.
 - rms_norm: per-head sum-of-squares via indicator matmul, sqrt on
   ACT, reciprocal on DVE, per-head broadcast on GPSIMD, and one
   scalar_tensor_tensor per head applies gamma*rstd straight out of
   PSUM.
 - Attention epilogue reciprocal reads PSUM directly; the out-proj
   matmuls of chunk nt are emitted after chunk nt+1's last PV matmul
   so the in-order PE queue always has ready work while the epilogue
   chain (DVE recip -> GPSIMD bcast -> DVE muls) completes.
 - Softmax denominator rides as a 65th column of V (ones, masked).
 - y is written as bf16 partials; the host sums the 8 cores.
"""

import numpy as np

P = 128
B = 2
N = 4096
M = 1024
C = 1024  # DIM == COND_DIM
H = 16
D = 64
HC = 2  # heads per core
CC = C // P  # contraction chunks
NT = N // 1024  # query chunks of 1024 (projection granularity)
NA = N // 512  # query chunks of 512 (attention granularity)
MC = M // P  # kv chunks of 128
VA = D + 1  # V columns + denominator column
EPS = 1e-6

_CACHE = {}


def _build():
    if "nc" in _CACHE:
        return _CACHE["nc"]

    import concourse.bass as bass  # noqa: F401
    import concourse.tile as tile
    from concourse import bacc, mybir

    f32 = mybir.dt.float32
    bf16 = mybir.dt.bfloat16
    AF = mybir.ActivationFunctionType
    MUL = mybir.AluOpType.mult

    nc = bacc.Bacc("TRN2", target_bir_lowering=False, debug=False, num_devices=8)

    xt_d = nc.dram_tensor("xt", [B, C, N], bf16, kind="ExternalInput").ap()
    ctxt_d = nc.dram_tensor("ctxt", [B, C, M], bf16, kind="ExternalInput").ap()
    wqt_d = nc.dram_tensor("wqt", [C, P], bf16, kind="ExternalInput").ap()
    wkt_d = nc.dram_tensor("wkt", [C, P], bf16, kind="ExternalInput").ap()
    wvt_d = nc.dram_tensor("wvt", [C, P], bf16, kind="ExternalInput").ap()
    wot_d = nc.dram_tensor("wot", [P, C], bf16, kind="ExternalInput").ap()
    brow_d = nc.dram_tensor("brow", [1, 3 * P], bf16, kind="ExternalInput").ap()
    gq_d = nc.dram_tensor("gqcol", [D, HC], f32, kind="ExternalInput").ap()
    gk_d = nc.dram_tensor("gkcol", [D, HC], f32, kind="ExternalInput").ap()
    ind2_d = nc.dram_tensor("ind2", [P, VA], bf16, kind="ExternalInput").ap()
    ident_d = nc.dram_tensor("ident", [P, P], f32, kind="ExternalInput").ap()
    mask_d = nc.dram_tensor("maskf", [P, B * MC], f32, kind="ExternalInput").ap()
    y_d = nc.dram_tensor("y", [B, N, C], bf16, kind="ExternalOutput").ap()

    with tile.TileContext(nc) as tc:
        with (
            tc.tile_pool(name="consts", bufs=1) as consts,
            tc.tile_pool(name="ctxp", bufs=2) as ctxp,
            tc.tile_pool(name="xp", bufs=16) as xp,
            tc.tile_pool(name="qk", bufs=1) as qk,
            tc.tile_pool(name="work", bufs=2) as work,
            tc.tile_pool(name="ptp", bufs=3) as ptp,
            tc.tile_pool(name="ps", bufs=1, space="PSUM") as ps,
        ):
            wq_sb = consts.tile([P, CC, P], bf16)
            nc.sync.dma_start(wq_sb[:], wqt_d.rearrange("(o p) m -> p o m", p=P))
            brow_sb = consts.tile([1, 3 * P], bf16)
            nc.sync.dma_start(brow_sb[:], brow_d[:])
            gq_sb = consts.tile([D, HC], f32)
            nc.sync.dma_start(gq_sb[:], gq_d[:])
            ind2_sb = consts.tile([P, VA], bf16)
            nc.sync.dma_start(ind2_sb[:], ind2_d[:])
            wk_sb = consts.tile([P, CC, P], bf16)
            nc.sync.dma_start(wk_sb[:], wkt_d.rearrange("(o p) m -> p o m", p=P))
            wv_sb = consts.tile([P, CC, P], bf16)
            nc.sync.dma_start(wv_sb[:], wvt_d.rearrange("(o p) m -> p o m", p=P))
            wo_sb = consts.tile([P, C], bf16)
            nc.sync.dma_start(wo_sb[:], wot_d[:])
            gk_sb = consts.tile([D, HC], f32)
            nc.sync.dma_start(gk_sb[:], gk_d[:])
            ident_sb = consts.tile([P, P], f32)
            nc.sync.dma_start(ident_sb[:], ident_d[:])
            mask_sb = consts.tile([P, B * MC], f32)
            nc.sync.dma_start(mask_sb[:], mask_d[:])
            eps_sb = consts.tile([VA, 1], f32)
            nc.vector.memset(eps_sb[:], EPS)
            ones_sb = consts.tile([1, 512], bf16)
            nc.vector.memset(ones_sb[:], 1.0)

            def proj(ps_dst, w_sb, src, brow):
                """ps_dst[128,1024] (psum) = w^T src + bias (seeded)."""
                for half in range(2):
                    hs = slice(half * 512, (half + 1) * 512)
                    nc.tensor.matmul(
                        ps_dst[:, hs], brow, ones_sb[:], start=True, stop=False
                    )
                    for cc in range(CC):
                        mv = src[cc][:, hs] if isinstance(src, list) else src[:, cc, hs]
                        nc.tensor.matmul(
                            ps_dst[:, hs],
                            w_sb[:, cc],
                            mv,
                            start=False,
                            stop=(cc == CC - 1),
                        )

            def rms_stage1(psrc):
                """Evacuate biased raw projections from PSUM (frees the
                bank) and square them.  Returns the in-flight state."""
                st = {}
                st["raw_a"] = work.tile([D, 1024], bf16, tag="raw_a", bufs=3, name="raw_a")
                nc.vector.tensor_copy(st["raw_a"][:], psrc[0:D, :])
                st["raw_b"] = work.tile([D, 1024], bf16, tag="raw_b", bufs=3, name="raw_b")
                nc.vector.tensor_copy(st["raw_b"][:], psrc[D:P, :])
                st["sq"] = work.tile([P, 1024], bf16, tag="sq", bufs=2, name="sq")
                nc.vector.tensor_mul(st["sq"][0:D, :], st["raw_a"][:], st["raw_a"][:])
                nc.vector.tensor_mul(st["sq"][D:P, :], st["raw_b"][:], st["raw_b"][:])
                return st

            def rms_stage2(st):
                """Per-head sum of squares -> rsqrt -> per-head broadcast."""
                ss = ps.tile([VA, 1024], f32, tag="O", bufs=2, name="ss")
                for half in range(2):
                    hs = slice(half * 512, (half + 1) * 512)
                    nc.tensor.matmul(
                        ss[:, hs], ind2_sb[:], st["sq"][:, hs], start=True, stop=True
                    )
                srt = work.tile([VA, 1024], f32, tag="srt", bufs=1, name="srt")
                nc.scalar.activation(
                    srt[:], ss[:], AF.Sqrt, scale=1.0 / D, bias=eps_sb[:]
                )
                rstd = work.tile([VA, 1024], f32, tag="rstd", bufs=2, name="rstd")
                nc.vector.reciprocal_approx_fast(out=rstd[:], in_=srt[:])
                r1 = work.tile([1, 1024], f32, tag="r1", bufs=2, name="r1")
                nc.vector.tensor_copy(r1[:], rstd[D : D + 1, :])
                st["rbc0"] = work.tile([D, 1024], f32, tag="rbc0", bufs=2, name="rbc0")
                nc.gpsimd.partition_broadcast(st["rbc0"][:], rstd[0:1, :])
                st["rbc1"] = work.tile([D, 1024], f32, tag="rbc1", bufs=2, name="rbc1")
                nc.gpsimd.partition_broadcast(st["rbc1"][:], r1[:])

            def rms_stage3(st):
                gcol, dst = st["gcol"], st["dst"]
                nc.vector.scalar_tensor_tensor(
                    out=dst[0:D, :], in0=st["raw_a"][:], scalar=gcol[:, 0:1],
                    in1=st["rbc0"][:], op0=MUL, op1=MUL,
                )
                nc.vector.scalar_tensor_tensor(
                    out=dst[D:P, :], in0=st["raw_b"][:], scalar=gcol[:, 1:2],
                    in1=st["rbc1"][:], op0=MUL, op1=MUL,
                )

            def rms_pump(q, force=False):
                """Software-pipeline the rms chains: stage2 runs one chunk
                behind stage1, stage3 two behind, so no engine queue ever
                waits head-of-line on a cross-engine dependency."""
                n = len(q)
                for idx, ent in enumerate(q):
                    age = n - idx
                    want = 3 if (age >= 3 or force) else (2 if age >= 2 else 1)
                    while ent["stage"] < want:
                        if ent["stage"] == 1:
                            rms_stage2(ent)
                        else:
                            rms_stage3(ent)
                        ent["stage"] += 1

            qtn = [
                qk.tile([P, 1024], bf16, tag=f"qtn{i}", name=f"qtn{i}")
                for i in range(B * NT)
            ]
            outp = qk.tile([P, N], bf16, tag="outtn")  # per-batch, reused

            def outproj_chunk(b, tc_, alternate=False):
                y_sb = work.tile([P, C], bf16, tag="ysb", bufs=3)
                ps_y = ps.tile([P, 1024], f32, tag="A", bufs=2)
                for ec in range(2):
                    nc.tensor.matmul(
                        ps_y[:, ec * 512 : (ec + 1) * 512],
                        outp[:, tc_ * P : (tc_ + 1) * P],
                        wo_sb[:, ec * 512 : (ec + 1) * 512],
                        start=True,
                        stop=True,
                    )
                if alternate and tc_ % 2 == 0:
                    nc.vector.tensor_copy(y_sb[:], ps_y[:])
                else:
                    nc.scalar.activation(y_sb[:], ps_y[:], AF.Copy)
                nc.scalar.dma_start(y_d[b, tc_ * P : (tc_ + 1) * P, :], y_sb[:])

            def load_xt_chunk(xt_r, nt):
                xt_cc = []
                for cc in range(CC):
                    t = xp.tile([P, 1024], bf16, tag="xt")
                    nc.sync.dma_start(
                        t[:], xt_r[:, cc, nt * 1024 : (nt + 1) * 1024]
                    )
                    xt_cc.append(t)
                return xt_cc

            def load_ctx(b):
                t = ctxp.tile([P, CC, M], bf16, tag="ctx")
                nc.sync.dma_start(
                    t[:], ctxt_d[b].rearrange("(o p) m -> p o m", p=P)
                )
                return t

            prefetched = {}
            outproj_todo = []
            for b in range(B):
                xt_r = xt_d[b].rearrange("(o p) n -> p o n", p=P)
                pf = prefetched.pop(b, None)
                ctx_sb = pf["ctx"] if pf else None

                def drain_outproj(k):
                    for _ in range(min(k, len(outproj_todo))):
                        outproj_chunk(*outproj_todo.pop(0))

                # ---- prologue: Q/K/V projections (pipelined rms),
                # interleaved with the previous batch's out-projection ----
                rq = []
                for nt in range(NT if pf is not None else 1):
                    if pf is not None and nt in pf["xt"]:
                        xt_cc = pf["xt"][nt]
                    else:
                        xt_cc = load_xt_chunk(xt_r, nt)
                    if ctx_sb is None:
                        ctx_sb = load_ctx(b)
                    ps_q = ps.tile([P, 1024], f32, tag="A", bufs=2)
                    proj(ps_q, wq_sb, xt_cc, brow_sb[:, 0:P])
                    st = rms_stage1(ps_q[:])
                    st.update(stage=1, gcol=gq_sb, dst=qtn[b * NT + nt][:])
                    rq.append(st)
                    rms_pump(rq)
                    drain_outproj(6)

                ktn = qk.tile([P, M], bf16, tag="ktn", bufs=2)
                ps_k = ps.tile([P, 1024], f32, tag="A", bufs=2)
                proj(ps_k, wk_sb, ctx_sb, brow_sb[:, P : 2 * P])
                st = rms_stage1(ps_k[:])
                st.update(stage=1, gcol=gk_sb, dst=ktn[:])
                rq.append(st)
                rms_pump(rq)
                drain_outproj(4)

                ps_v = ps.tile([P, 1024], f32, tag="A", bufs=2)
                proj(ps_v, wv_sb, ctx_sb, brow_sb[:, 2 * P : 3 * P])
                rms_pump(rq, force=True)
                vt_sb = work.tile([P, M], f32, tag="vt", bufs=2)
                nc.vector.tensor_copy(vt_sb[:], ps_v[:])
                drain_outproj(4)
                vaug = qk.tile([P, MC, 2 * VA], bf16, tag="vaug", bufs=2)
                for mc in range(MC):
                    ps_t = ps.tile([P, P], f32, tag="O", bufs=2)
                    nc.tensor.transpose(
                        ps_t[:], vt_sb[:, mc * P : (mc + 1) * P], ident_sb[:]
                    )
                    mcol = mask_sb[:, b * MC + mc : b * MC + mc + 1]
                    nc.vector.tensor_mul(
                        vaug[:, mc, 0:D], ps_t[:, 0:D], mcol.to_broadcast((P, D))
                    )
                    nc.vector.tensor_copy(vaug[:, mc, D : D + 1], mcol)
                    nc.vector.tensor_mul(
                        vaug[:, mc, VA : VA + D],
                        ps_t[:, D : 2 * D],
                        mcol.to_broadcast((P, D)),
                    )
                    nc.vector.tensor_copy(vaug[:, mc, VA + D : VA + D + 1], mcol)
                drain_outproj(len(outproj_todo))

                # ---- attention ----
                pending_xt = {}
                if pf is None:
                    pending_xt[1] = load_xt_chunk(xt_r, 1)
                for na in range(NA):
                    if pf is None and na in (1, 3, 5):
                        nt = (na + 1) // 2
                        if nt + 1 < NT:
                            pending_xt[nt + 1] = load_xt_chunk(xt_r, nt + 1)
                        ps_q2 = ps.tile([P, 1024], f32, tag="O", bufs=2, name="ps_q2")
                        proj(ps_q2, wq_sb, pending_xt.pop(nt), brow_sb[:, 0:P])
                        st2 = rms_stage1(ps_q2[:])
                        st2.update(stage=1, gcol=gq_sb, dst=qtn[b * NT + nt][:])
                        rms_stage2(st2)
                        rms_stage3(st2)
                    if na == 6 and b + 1 < B:
                        xt_r2 = xt_d[b + 1].rearrange("(o p) n -> p o n", p=P)
                        prefetched[b + 1] = {
                            "xt": {nt: load_xt_chunk(xt_r2, nt) for nt in (0, 1)},
                            "ctx": load_ctx(b + 1),
                        }
                    nsl = slice(na * 512, (na + 1) * 512)
                    qt = qtn[b * NT + na // 2]
                    qsl = slice((na % 2) * 512, (na % 2) * 512 + 512)
                    ps_o = ps.tile([VA, 1024], f32, tag="O", bufs=2, name="ps_o")
                    for mc in range(MC):
                        msl = slice(mc * P, (mc + 1) * P)
                        ps_s = ps.tile([P, 1024], f32, tag="A", bufs=2)
                        for h in range(2):
                            hsl = slice(D * h, D * (h + 1))
                            nc.tensor.matmul(
                                ps_s[:, h * 512 : (h + 1) * 512],
                                ktn[hsl, msl],
                                qt[hsl, qsl],
                                start=True,
                                stop=True,
                            )
                        pt = ptp.tile([P, 1024], bf16, tag="pt")
                        nc.scalar.activation(pt[:], ps_s[:], AF.Exp)
                        for h in range(2):
                            nc.tensor.matmul(
                                ps_o[:, h * 512 : (h + 1) * 512],
                                vaug[:, mc, h * VA : (h + 1) * VA],
                                pt[:, h * 512 : (h + 1) * 512],
                                start=(mc == 0),
                                stop=(mc == MC - 1),
                            )
                    # epilogue: divide by the denominator row (64) per head
                    den = work.tile([1, 1024], f32, tag="den", name="den", bufs=1)
                    nc.vector.tensor_copy(den[:], ps_o[D : D + 1, :])
                    rec = work.tile([1, 1024], f32, tag="rec", name="rec", bufs=1)
                    nc.vector.reciprocal_approx_fast(out=rec[:], in_=den[:])
                    rbc2 = work.tile([D, 1024], f32, tag="rbc2", name="rbc2", bufs=1)
                    nc.gpsimd.partition_broadcast(rbc2[:], rec[:])
                    nc.vector.scalar_tensor_tensor(
                        out=outp[0:D, nsl], in0=ps_o[0:D, 0:512], scalar=1.0,
                        in1=rbc2[:, 0:512], op0=MUL, op1=MUL,
                    )
                    nc.vector.scalar_tensor_tensor(
                        out=outp[D:P, nsl], in0=ps_o[0:D, 512:1024], scalar=1.0,
                        in1=rbc2[:, 512:1024], op0=MUL, op1=MUL,
                    )

                # ---- queue this batch's out-projection; the last batch
                # drains it immediately (tail) ----
                outproj_todo = [(b, tc_) for tc_ in range(N // P)]
                if b == B - 1:
                    for ent in outproj_todo:
                        outproj_chunk(*ent, alternate=True)
                    outproj_todo = []

    nc.compile()
    _CACHE["nc"] = nc
    return nc


def _make_in_maps(x, context, context_mask, Wq, bq, Wkv, bkv, gq, gk, Wo, bo):
    import ml_dtypes

    f32 = np.float32
    bf16 = ml_dtypes.bfloat16
    xt = np.ascontiguousarray(np.transpose(x, (0, 2, 1))).astype(bf16)
    ctxt = np.ascontiguousarray(np.transpose(context, (0, 2, 1))).astype(bf16)
    # maskf[p, b*MC + mc] = mask[b, mc*128 + p]
    maskf = np.ascontiguousarray(
        np.transpose(
            np.asarray(context_mask, dtype=f32).reshape(B, MC, P), (2, 0, 1)
        ).reshape(P, B * MC)
    )
    ident = np.eye(P, dtype=f32)
    ind2 = np.zeros((P, VA), dtype=f32)
    ind2[0:D, 0] = 1.0
    ind2[D : 2 * D, D] = 1.0
    ind2 = ind2.astype(bf16)

    in_maps = []
    for c in range(8):
        hs = slice(P * c, P * (c + 1))
        gqcol = np.zeros((D, HC), dtype=f32)
        gkcol = np.zeros((D, HC), dtype=f32)
        for h in range(HC):
            gqcol[:, h] = gq[HC * c + h] * (1.0 / np.sqrt(D))
            gkcol[:, h] = gk[HC * c + h]
        brow = np.zeros((1, 3 * P), dtype=f32)
        brow[0, 0:P] = np.asarray(bq[hs], dtype=f32)
        brow[0, P : 2 * P] = np.asarray(bkv[hs], dtype=f32)
        brow[0, 2 * P : 3 * P] = np.asarray(
            bkv[C + P * c : C + P * (c + 1)], dtype=f32
        )
        in_maps.append(
            {
                "xt": xt,
                "ctxt": ctxt,
                "wqt": np.ascontiguousarray(Wq[hs].T).astype(bf16),
                "wkt": np.ascontiguousarray(Wkv[hs].T).astype(bf16),
                "wvt": np.ascontiguousarray(
                    Wkv[C + P * c : C + P * (c + 1)].T
                ).astype(bf16),
                "wot": np.ascontiguousarray(Wo[:, hs].T).astype(bf16),
                "brow": brow.astype(bf16),
                "gqcol": gqcol,
                "gkcol": gkcol,
                "ind2": ind2,
                "ident": ident,
                "maskf": maskf,
            }
        )
    return in_maps


def _run(in_maps, **spmd_kwargs):
    from concourse import bass_utils

    nc = _build()
    return bass_utils.run_bass_kernel_spmd(
        nc, in_maps, core_ids=list(range(8)), **spmd_kwargs
    )


def kernel(x, context, context_mask, Wq, bq, Wkv, bkv, gq, gk, Wo, bo):
    in_maps = _make_in_maps(
        x, context, context_mask, Wq, bq, Wkv, bkv, gq, gk, Wo, bo
    )
    res = _run(in_maps)
    y = np.zeros((B, N, C), dtype=np.float64)
    for c in range(8):
        y += np.asarray(res.results[c]["y"], dtype=np.float64)
    y += np.asarray(bo, dtype=np.float64)
    return y.astype(np.float32)
